# revision 24
# baseline (speedup 1.0000x reference)
"""Multi-head self-attention Trainium2 kernel (8 NeuronCores, head-parallel).

Problem: L=4096, F_IN=1024, H=16, DH=64, F_OUT=1024, fp32.
Sharding: 2 heads per core (tensor parallel over heads). Each core computes
its 2 heads' attention and its partial output projection; the host sums the
8 partials (the all-reduce of the sharding hint, done at gather time).

Numerics: x and Wq/Wk/Wv are loaded in bf16; projections accumulate in fp32
PSUM. The attention matmuls run in bf16 with fp32 PSUM accumulation. Exps
alternate between ScalarE (exact ACT exp) and VectorE (Schraudolph
int16-bitcast approximation); the softmax denominator is summed from the
*rounded* attention weights (ones-column trick), so rounding largely
cancels in the normalization.

Schedule design (from perfetto analysis):
  - The binding constraint is a latency loop: scores(n+k) cannot issue
    until the PSUM bank of scores(n) is freed by exp(n). Phase 2 uses a
    persistent 5-bank scores arena with slots rotating mod 5 (2.5 tiles in
    flight) so the loop latency (scores + sem + exp + sem ~ 1.6us) divides
    by 2.5 and the PE becomes the pacer. Chunk 0 uses a 4-slot aligned
    rotation because phase 1 still holds 2 PSUM banks.
  - Per-jt emission order: exp first, then long-ready PE work (attn@v
    lagged 16 tiles, out-proj of the previous chunk), and the bank-gated
    scores LAST so a stall cannot head-of-line-block ready matmuls.
  - Chunk-end work (final attn@v block, pv evac, norm) is deferred into
    the next chunk's first iterations.
  - Input DMAs are spread across HW queues (sync/vector/gpsimd) and the
    x quarters are fetched half-first so the first k-projection can start
    as early as possible; a burst of dummy matmuls warms the PE HAM clock
    gate during the initial DMA wait.
"""

import numpy as np

L, F_IN, H, DH, F_OUT = 4096, 1024, 16, 64, 1024

# Schraudolph exp constants (DVE): int16(ps*SCH_A + SCH_B) bitcast bf16
SCH_C = 0.0579
SCH_A = 128.0 * 1.4426950408889634 * 0.125
SCH_B = 128.0 * (127.0 - SCH_C)
NCORES = 8
HPC = H // NCORES  # heads per core = 2
D2 = HPC * DH      # 128, per-core packed head dim

_BUILT = None


def _build():
    import os

    import concourse.bass as bass  # noqa: F401
    import concourse.mybir as mybir
    import concourse.tile as tile
    from concourse import bacc
    from concourse.masks import make_identity

    F = mybir.dt.float32
    FR = mybir.dt.float32r
    BF = mybir.dt.bfloat16
    I16 = mybir.dt.int16
    Alu = mybir.AluOpType
    Act = mybir.ActivationFunctionType

    nc = bacc.Bacc("TRN2", target_bir_lowering=False, debug=False)

    xT_d = nc.declare_dram_parameter("xT", [F_IN, L], BF, isOutput=False)
    wq_d = nc.declare_dram_parameter("wq", [F_IN, D2], BF, isOutput=False)
    wk_d = nc.declare_dram_parameter("wk", [F_IN, D2], BF, isOutput=False)
    wv_d = nc.declare_dram_parameter("wv", [F_IN, D2], BF, isOutput=False)
    bq_d = nc.declare_dram_parameter("bq", [D2], F, isOutput=False)
    bk_d = nc.declare_dram_parameter("bk", [D2], F, isOutput=False)
    wo0_d = nc.declare_dram_parameter("wo0", [DH, F_OUT], F, isOutput=False)
    wo1_d = nc.declare_dram_parameter("wo1", [DH, F_OUT], F, isOutput=False)
    out_d = nc.declare_dram_parameter("out", [L, F_OUT], F, isOutput=True)

    dbg = bool(os.environ.get("K_DEBUG"))
    if dbg:
        dbg_q = nc.declare_dram_parameter("dbg_q", [128, L], F, isOutput=True)
        dbg_k = nc.declare_dram_parameter("dbg_k", [128, L], F, isOutput=True)
        dbg_v = nc.declare_dram_parameter("dbg_v", [128, 32 * 65], F, isOutput=True)

    KT = F_IN // 128   # 8 f-tiles
    NI = L // 512      # 8 i-chunks
    NJ = L // 128      # 32 j-tiles
    QL = 1024          # quarter width in L
    NQ = L // QL       # 4 quarters

    with tile.TileContext(nc) as tc:
        with tc.tile_pool(name="persist", bufs=1) as pp:
            qT = pp.tile([128, L], BF, tag="qT")             # [d2, i]
            kT = pp.tile([128, L], BF, tag="kT")             # [d2, j]
            vx0 = pp.tile([128, NJ, DH + 1], BF, tag="vx0")  # [j_in, jt, d|1]
            vx1 = pp.tile([128, NJ, DH + 1], BF, tag="vx1")
            bq = pp.tile([128, 1], F, tag="bq")
            bk = pp.tile([128, 1], F, tag="bk")
            ones32 = pp.tile([128, NJ], F, tag="ones32")
            warm = pp.tile([1, 1], F, tag="warm")
            wmm = pp.tile([64, 128], BF, tag="wmm")

            # pre-warm the exp table set while DMAs run
            nc.vector.memset(warm[:], 0.0)
            nc.scalar.activation(warm[:], warm[:], Act.Exp, scale=1.0)

            nc.vector.memset(wmm[:], 0.0)
            nc.vector.memset(ones32[:], 1.0)
            nc.vector.tensor_copy(vx0[:, :, DH:DH + 1], ones32[:, :, None])
            nc.vector.tensor_copy(vx1[:, :, DH:DH + 1], ones32[:, :, None])

            # Pools for the attention phase are opened before phase 1 is
            # emitted so the scheduler can overlap the phase-1 tail with
            # early score matmuls (PSUM: arena 5 + pv 2 + {ps1 2 during
            # phase1 / pso 1 after} = 8; chunk 0 only touches arena
            # slots 0-3).
            with tc.tile_pool(name="p2", bufs=1) as p2, \
                 tc.tile_pool(name="p2v", bufs=2) as p2v, \
                 tc.tile_pool(name="expp", bufs=20) as pe, \
                 tc.tile_pool(name="outp", bufs=4) as po, \
                 tc.tile_pool(name="ps2s", bufs=1, space="PSUM") as ps2s, \
                 tc.tile_pool(name="ps2v", bufs=1, space="PSUM") as ps2v:
                pss0 = ps2s.tile([128, 1024], F, tag="pss0")
                pss1 = ps2s.tile([128, 1024], F, tag="pss1")
                pv0 = ps2v.tile([128, 512], F, tag="pv0")
                pv1 = ps2v.tile([128, 512], F, tag="pv1")
                wo0 = p2.tile([DH, F_OUT], FR, tag="wo0")
                wo1 = p2.tile([DH, F_OUT], FR, tag="wo1")

                # ---- Phase 1: QKV projections over 4 quarters of L ----
                with tc.tile_pool(name="p1w", bufs=1) as p1w, \
                     tc.tile_pool(name="p1x", bufs=2) as p1x, \
                     tc.tile_pool(name="ps1", bufs=2, space="PSUM") as ps1:
                    wq = p1w.tile([128, KT, D2], BF, tag="wq")
                    wk = p1w.tile([128, KT, D2], BF, tag="wk")
                    wv = p1w.tile([128, KT, D2], BF, tag="wv")
                    ident = p1w.tile([128, 128], F, tag="ident")
                    for wt, wd in ((wk, wk_d), (wv, wv_d), (wq, wq_d)):
                        nc.scalar.dma_start(
                            out=wt[:],
                            in_=wd.ap().rearrange("(k p) d -> p k d", p=128),
                        )
                    make_identity(nc, ident[:])
                    nc.scalar.dma_start(out=bq[:], in_=bq_d.ap()[:, None])
                    nc.scalar.dma_start(out=bk[:], in_=bk_d.ap()[:, None])
                    nc.scalar.dma_start(out=wo0[:],
                                        in_=wo0_d.ap().bitcast(FR))
                    nc.scalar.dma_start(out=wo1[:],
                                        in_=wo1_d.ap().bitcast(FR))

                    # HAM warm-up: ~4us of dummy matmuls during the input
                    # DMA wait so the real projections run at 2.4 GHz
                    psw = ps1.tile([128, 512], F, tag="ps1")
                    for _ in range(80):
                        nc.tensor.matmul(
                            psw[0:64, 0:128], wmm[:, 0:64], wmm[:, :],
                            start=True, stop=True,
                        )

                    def proj(wt, dst, bias, xt, c0, g0):
                        ps = ps1.tile([128, 512], F, tag="ps1")
                        for kt in range(KT):
                            nc.tensor.matmul(
                                ps[:], wt[:, kt, :], xt[:, kt, c0:c0 + 512],
                                start=(kt == 0), stop=(kt == KT - 1),
                            )
                        if bias is not None:
                            nc.scalar.activation(
                                dst[:, g0:g0 + 512], ps[:], Act.Identity,
                                bias=bias[:], scale=1.0,
                            )
                        else:
                            nc.scalar.copy(dst[:, c0:c0 + 512], ps[:])

                    q_tiles = {}

                    def emit_q_dma(qq):
                        l0 = qq * QL
                        xt = p1x.tile([128, KT, QL], BF, tag="xt")
                        # per-kt pieces so each projection matmul can run
                        # as soon as its f-tile lands; halves split over
                        # two queues
                        for kt in range(KT):
                            nc.sync.dma_start(
                                out=xt[:, kt, 0:QL // 2],
                                in_=xT_d.ap()[kt * 128:(kt + 1) * 128,
                                              l0:l0 + QL // 2])
                        for kt in range(KT):
                            nc.gpsimd.dma_start(
                                out=xt[:, kt, QL // 2:QL],
                                in_=xT_d.ap()[kt * 128:(kt + 1) * 128,
                                              l0 + QL // 2:l0 + QL])
                        q_tiles[qq] = xt

                    def emit_q_comp(qq):
                        l0 = qq * QL
                        xt = q_tiles.pop(qq)
                        vTq = p1x.tile([128, QL], F, tag="vTq")
                        for ch in range(QL // 512):
                            proj(wk, kT, bk, xt, ch * 512, l0 + ch * 512)
                            proj(wv, vTq, None, xt, ch * 512, ch * 512)
                            if qq == 0:
                                proj(wq, qT, bq, xt, ch * 512, l0 + ch * 512)
                        for jl in range(QL // 128):
                            jt = qq * (QL // 128) + jl
                            pt = ps1.tile([128, 512], F, tag="ps1")
                            nc.tensor.transpose(
                                pt[:, 0:128],
                                vTq[:, jl * 128:(jl + 1) * 128], ident[:])
                            nc.vector.tensor_copy(vx0[:, jt, 0:DH], pt[:, 0:DH])
                            nc.vector.tensor_copy(vx1[:, jt, 0:DH],
                                                  pt[:, DH:D2])
                        if qq != 0:
                            for ch in range(QL // 512):
                                proj(wq, qT, bq, xt, ch * 512, l0 + ch * 512)

                    # chunk 0's attention interleaves into the remaining
                    # quarters so the in-order PE queue no longer serializes
                    # all of phase 1 ahead of the first scores matmul
                    emit_q_dma(0)
                    emit_q_comp(0)
                    emit_q_dma(1)
                    c0 = _phase2_chunk0(nc, (pss0, pss1), pv0, pv1, pe,
                                        p2v, qT, kT, vx0, vx1, NJ, F, FR,
                                        BF, I16, Act, Alu, emit_q_dma,
                                        emit_q_comp)

                if dbg:
                    nc.sync.dma_start(out=dbg_q.ap(), in_=qT[:].bitcast(F))
                    nc.sync.dma_start(out=dbg_k.ap(), in_=kT[:].bitcast(F))
                    nc.sync.dma_start(
                        out=dbg_v.ap(),
                        in_=vx0[:].bitcast(F).rearrange("p a b -> p (a b)"))

                # ---- Phase 2+3: attention, interleaved normalize/out-proj ----
                with tc.tile_pool(name="ps2o", bufs=2, space="PSUM") as ps2o:
                    _phase2(nc, (pss0, pss1), pv0, pv1, ps2o, pe, po, p2v,
                            qT, kT, vx0, vx1, wo0, wo1, out_d,
                            NI, NJ, F, FR, BF, I16, Act, Alu, c0)

    nc.compile()
    return nc


def _score_tile(tiles, n, NJ):
    return tiles[n % 2]


def _mk_emit_scores(nc, tiles, qT, kT, NJ):
    def emit_scores(n):
        ic, jt = n // NJ, n % NJ
        i0 = ic * 512
        j0 = jt * 128
        ps = _score_tile(tiles, n, NJ)
        nc.tensor.matmul(
            ps[:, 0:512], kT[0:64, j0:j0 + 128], qT[0:64, i0:i0 + 512],
            start=True, stop=True, tile_position=(0, 0),
        )
        nc.tensor.matmul(
            ps[:, 512:1024], kT[64:128, j0:j0 + 128],
            qT[64:128, i0:i0 + 512],
            start=True, stop=True, tile_position=(64, 0),
        )
    return emit_scores


def _mk_emit_exp(nc, tiles, NJ, BF, I16, Act, Alu):
    # Each tile's exp runs as two concurrent half-ops, one per engine
    # (different PSUM banks), so the scores tile is freed after ~0.7us
    # instead of ~1.2us — the B=2 pipeline becomes engine-bound instead
    # of latency-bound.  Heads alternate engines for error symmetry.
    def emit_exp(n, jt, eT):
        ps = _score_tile(tiles, n, NJ)
        a, b = (0, 512) if jt % 2 == 0 else (512, 0)
        nc.scalar.activation(eT[:, a:a + 512], ps[:, a:a + 512],
                             Act.Exp, scale=0.125)
        nc.vector.tensor_scalar(
            eT[:, b:b + 512].bitcast(I16), ps[:, b:b + 512],
            SCH_A, SCH_B, Alu.mult, Alu.add)
    return emit_exp


def _mk_emit_chain(nc, pv0, pv1, vx0, vx1, NJ):
    """One 4-tile attn@v accumulation chain for one head.  Chains are
    spread ~evenly over the j-loop (the PE activity monitor re-throttles
    the clock if any ~3.4us window is mostly idle)."""
    def emit_chain(eTs, b0, head):
        pv, vx, c0 = ((pv0, vx0, 0) if head == 0 else (pv1, vx1, 512))
        for bjt in range(b0, b0 + 4):
            nc.tensor.matmul(
                pv[0:DH + 1, :], vx[:, bjt, :],
                eTs[bjt][:, c0:c0 + 512],
                start=(bjt == 0), stop=(bjt == NJ - 1),
            )
    return emit_chain


def _phase2(nc, stiles, pv0, pv1, ps2o, pe, po, p2v, qT, kT, vx0, vx1,
            wo0, wo1, out_d, NI, NJ, F, FR, BF, I16, Act, Alu, c0):
    emit_scores = _mk_emit_scores(nc, stiles, qT, kT, NJ)
    emit_exp = _mk_emit_exp(nc, stiles, NJ, BF, I16, Act, Alu)
    emit_chain = _mk_emit_chain(nc, pv0, pv1, vx0, vx1, NJ)

    def norm_unit(ic, p0, p1):
        # reciprocal + broadcast of the softmax denominators for chunk ic
        for (va, _), tg in ((p0, "0"), (p1, "1")):
            sh = p2v.tile([1, 512], F, tag="sh" + tg)
            rc = p2v.tile([1, 512], F, tag="rc" + tg)
            rb = p2v.tile([DH, 512], F, tag="rb" + tg)
            nc.sync.dma_start(out=sh[:], in_=va[DH:DH + 1, :].bitcast(F))
            nc.vector.reciprocal_approx_fast(out=rc[:], in_=sh[:])
            nc.gpsimd.partition_broadcast(rb[:], rc[:], channels=DH)
            nc.vector.tensor_mul(va[0:DH, :], va[0:DH, :], rb[:])

    def oproj_unit(ic, p0, p1, iw, fc, evac="s"):
        # one output-projection tile of chunk ic
        isl = slice(iw * 128, (iw + 1) * 128)
        r0 = ic * 512 + iw * 128
        f0 = fc * 512
        pso = ps2o.tile([128, 512], F, tag="pso")
        nc.tensor.matmul(
            pso[:], p0[0][0:DH, isl], wo0[:, f0:f0 + 512],
            start=True, stop=False,
        )
        nc.tensor.matmul(
            pso[:], p1[0][0:DH, isl], wo1[:, f0:f0 + 512],
            start=False, stop=True,
        )
        ot = po.tile([128, 512], F, tag="ot")
        if evac == "s":
            nc.scalar.copy(ot[:], pso[:])
            nc.sync.dma_start(
                out=out_d.ap()[r0:r0 + 128, f0:f0 + 512], in_=ot[:])
        else:
            nc.vector.tensor_copy(ot[:], pso[:])
            nc.gpsimd.dma_start(
                out=out_d.ap()[r0:r0 + 128, f0:f0 + 512], in_=ot[:])

    # attn@v chains at jts 5,6, 9,10, ..., 29,30 (b=(jt-5)//4, head par),
    # out-proj units of the previous chunk at the chain-free jts
    OPROJ_JTS = (7, 8, 11, 12, 15, 16, 19, 20)

    pending = c0["pending"]
    carry = c0["carry"]
    for ic in range(1, NI):
        units = []
        if pending is not None:
            pic, pp0, pp1 = pending[0], pending[1], pending[2]
            units = [(pic, pp0, pp1, iw, fc,
                      "v" if (iw * 2 + fc) % 3 == 2 else "s")
                     for iw in range(4) for fc in range(F_OUT // 512)]
        eTs = []
        for jt in range(NJ):
            n = ic * NJ + jt
            eT = pe.tile([128, 1024], BF, tag="eT")
            emit_exp(n, jt, eT)
            eTs.append(eT)
            if jt == 1 and carry is not None:
                carry(0)
            if jt == 2 and pending is not None:
                nc.vector.tensor_copy(pending[1][0][:], pv0[0:DH + 1, :])
            if jt == 3 and carry is not None:
                carry(1)
                carry = None
                nc.vector.tensor_copy(pending[2][0][:], pv1[0:DH + 1, :])
            if jt == 4 and pending is not None:
                norm_unit(pending[0], pending[1], pending[2])
            if jt >= 5 and (jt - 5) % 4 in (0, 1):
                emit_chain(eTs, 4 * ((jt - 5) // 4), (jt - 5) % 4)
            if jt in OPROJ_JTS and units:
                oproj_unit(*units.pop(0))
            if n + 1 < NI * NJ:
                emit_scores(n + 1)
        va0 = p2v.tile([DH + 1, 512], FR, tag="va0")
        va1 = p2v.tile([DH + 1, 512], FR, tag="va1")
        if ic < NI - 1:
            ceTs = eTs
            carry = (lambda head, e=ceTs: emit_chain(e, NJ - 4, head))
            pending = (ic, (va0, None), (va1, None))
        else:
            emit_chain(eTs, NJ - 4, 0)
            emit_chain(eTs, NJ - 4, 1)
            nc.vector.tensor_copy(va0[:], pv0[0:DH + 1, :])
            nc.vector.tensor_copy(va1[:], pv1[0:DH + 1, :])
            pending = (ic, (va0, None), (va1, None))

    norm_unit(pending[0], pending[1], pending[2])
    for iw in range(4):
        for fc in range(F_OUT // 512):
            oproj_unit(pending[0], pending[1], pending[2], iw, fc,
                       "s" if fc == 0 else "v")


def _phase2_chunk0(nc, stiles, pv0, pv1, pe, p2v, qT, kT, vx0, vx1, NJ,
                   F, FR, BF, I16, Act, Alu, emit_q_dma, emit_q_comp):
    """Chunk 0's j-loop, emitted inside the phase-1 pool scope with the
    remaining quarters' DMAs/compute interleaved at fixed j-slots."""
    emit_scores = _mk_emit_scores(nc, stiles, qT, kT, NJ)
    emit_exp = _mk_emit_exp(nc, stiles, NJ, BF, I16, Act, Alu)
    emit_chain = _mk_emit_chain(nc, pv0, pv1, vx0, vx1, NJ)

    eTs = []
    emit_scores(0)
    for jt in range(NJ):
        eT = pe.tile([128, 1024], BF, tag="eT")
        emit_exp(jt, jt, eT)
        eTs.append(eT)
        if jt >= 5 and (jt - 5) % 4 in (0, 1):
            emit_chain(eTs, 4 * ((jt - 5) // 4), (jt - 5) % 4)
        if jt % 8 == 6 and jt < NJ - 2:
            emit_q_comp(jt // 8 + 1)
            if jt // 8 + 2 < 4:
                emit_q_dma(jt // 8 + 2)
        emit_scores(jt + 1)
    va0 = p2v.tile([DH + 1, 512], FR, tag="va0")
    va1 = p2v.tile([DH + 1, 512], FR, tag="va1")
    carry = (lambda head, e=eTs: emit_chain(e, NJ - 4, head))
    return {"pending": (0, (va0, None), (va1, None)), "carry": carry}


def _get_built():
    global _BUILT
    if _BUILT is None:
        _BUILT = _build()
    return _BUILT


def kernel(x, Wq, bq, Wk, bk, Wv, bv, Wo, bo):
    from concourse.bass_utils import run_bass_kernel_spmd

    x = np.ascontiguousarray(np.asarray(x, dtype=np.float32))
    Wq = np.asarray(Wq, dtype=np.float32)
    Wk = np.asarray(Wk, dtype=np.float32)
    Wv = np.asarray(Wv, dtype=np.float32)
    Wo = np.asarray(Wo, dtype=np.float32)
    bq = np.asarray(bq, dtype=np.float32)
    bk = np.asarray(bk, dtype=np.float32)
    bv = np.asarray(bv, dtype=np.float32)
    bo = np.asarray(bo, dtype=np.float32)

    nc = _get_built()

    import ml_dtypes
    BFH = ml_dtypes.bfloat16
    xT = np.ascontiguousarray(x.T.astype(BFH))  # [F_IN, L] bf16
    in_maps = []
    for c in range(NCORES):
        hs = slice(c * HPC, (c + 1) * HPC)
        in_maps.append({
            "xT": xT,
            "wq": np.ascontiguousarray(
                Wq[:, hs, :].reshape(F_IN, D2).astype(BFH)),
            "wk": np.ascontiguousarray(
                Wk[:, hs, :].reshape(F_IN, D2).astype(BFH)),
            "wv": np.ascontiguousarray(
                Wv[:, hs, :].reshape(F_IN, D2).astype(BFH)),
            "bq": np.ascontiguousarray(bq[hs].reshape(D2)),
            "bk": np.ascontiguousarray(bk[hs].reshape(D2)),
            "wo0": np.ascontiguousarray(Wo[c * HPC]),
            "wo1": np.ascontiguousarray(Wo[c * HPC + 1]),
        })

    res = run_bass_kernel_spmd(nc, in_maps, list(range(NCORES)))
    acc = np.zeros((L, F_OUT), dtype=np.float64)
    for c in range(NCORES):
        acc += res.results[c]["out"].astype(np.float64)
    # bv contribution (softmax rows sum to 1) + bo, both exact on host
    acc += (bv.reshape(1, H * DH).astype(np.float64)
            @ Wo.reshape(H * DH, F_OUT).astype(np.float64))
    acc += bo.astype(np.float64)
    return acc.astype(np.float32)


# revision 25
# speedup vs baseline: 1.0957x; 1.0957x over previous
"""Multi-head self-attention Trainium2 kernel (8 NeuronCores, head-parallel).

Problem: L=4096, F_IN=1024, H=16, DH=64, F_OUT=1024, fp32.
Sharding: 2 heads per core (tensor parallel over heads). Each core computes
its 2 heads' attention and its partial output projection; the host sums the
8 partials (the all-reduce of the sharding hint, done at gather time).

Numerics: x and Wq/Wk/Wv are loaded in bf16; projections accumulate in fp32
PSUM. The attention matmuls run in bf16 with fp32 PSUM accumulation. Exps
alternate between ScalarE (exact ACT exp) and VectorE (Schraudolph
int16-bitcast approximation); the softmax denominator is summed from the
*rounded* attention weights (ones-column trick), so rounding largely
cancels in the normalization.

Schedule design (from perfetto analysis):
  - The binding constraint is a latency loop: scores(n+k) cannot issue
    until the PSUM bank of scores(n) is freed by exp(n). Phase 2 uses a
    persistent 5-bank scores arena with slots rotating mod 5 (2.5 tiles in
    flight) so the loop latency (scores + sem + exp + sem ~ 1.6us) divides
    by 2.5 and the PE becomes the pacer. Chunk 0 uses a 4-slot aligned
    rotation because phase 1 still holds 2 PSUM banks.
  - Per-jt emission order: exp first, then long-ready PE work (attn@v
    lagged 16 tiles, out-proj of the previous chunk), and the bank-gated
    scores LAST so a stall cannot head-of-line-block ready matmuls.
  - Chunk-end work (final attn@v block, pv evac, norm) is deferred into
    the next chunk's first iterations.
  - Input DMAs are spread across HW queues (sync/vector/gpsimd) and the
    x quarters are fetched half-first so the first k-projection can start
    as early as possible; a burst of dummy matmuls warms the PE HAM clock
    gate during the initial DMA wait.
"""

import numpy as np

L, F_IN, H, DH, F_OUT = 4096, 1024, 16, 64, 1024

# Schraudolph exp constants (DVE): int16(ps*SCH_A + SCH_B) bitcast bf16
SCH_C = 0.0579
SCH_A = 128.0 * 1.4426950408889634 * 0.125
SCH_B = 128.0 * (127.0 - SCH_C)
NCORES = 8
HPC = H // NCORES  # heads per core = 2
D2 = HPC * DH      # 128, per-core packed head dim

_BUILT = None


def _build():
    import os

    import concourse.bass as bass  # noqa: F401
    import concourse.mybir as mybir
    import concourse.tile as tile
    from concourse import bacc
    from concourse.masks import make_identity

    F = mybir.dt.float32
    FR = mybir.dt.float32r
    BF = mybir.dt.bfloat16
    I16 = mybir.dt.int16
    Alu = mybir.AluOpType
    Act = mybir.ActivationFunctionType

    nc = bacc.Bacc("TRN2", target_bir_lowering=False, debug=False)

    xT_d = nc.declare_dram_parameter("xT", [F_IN, L], BF, isOutput=False)
    wq_d = nc.declare_dram_parameter("wq", [F_IN, D2], BF, isOutput=False)
    wk_d = nc.declare_dram_parameter("wk", [F_IN, D2], BF, isOutput=False)
    wv_d = nc.declare_dram_parameter("wv", [F_IN, D2], BF, isOutput=False)
    bq_d = nc.declare_dram_parameter("bq", [D2], F, isOutput=False)
    bk_d = nc.declare_dram_parameter("bk", [D2], F, isOutput=False)
    wo0_d = nc.declare_dram_parameter("wo0", [DH, F_OUT], F, isOutput=False)
    wo1_d = nc.declare_dram_parameter("wo1", [DH, F_OUT], F, isOutput=False)
    out_d = nc.declare_dram_parameter("out", [L, F_OUT], F, isOutput=True)

    dbg = bool(os.environ.get("K_DEBUG"))
    if dbg:
        dbg_q = nc.declare_dram_parameter("dbg_q", [128, L], F, isOutput=True)
        dbg_k = nc.declare_dram_parameter("dbg_k", [128, L], F, isOutput=True)
        dbg_v = nc.declare_dram_parameter("dbg_v", [128, 32 * 65], F, isOutput=True)

    KT = F_IN // 128   # 8 f-tiles
    NI = L // 512      # 8 i-chunks
    NJ = L // 128      # 32 j-tiles
    QL = 1024          # quarter width in L
    NQ = L // QL       # 4 quarters

    with tile.TileContext(nc) as tc:
        with tc.tile_pool(name="persist", bufs=1) as pp:
            qT = pp.tile([128, L], BF, tag="qT")             # [d2, i]
            kT = pp.tile([128, L], BF, tag="kT")             # [d2, j]
            vx0 = pp.tile([128, NJ, DH + 1], BF, tag="vx0")  # [j_in, jt, d|1]
            vx1 = pp.tile([128, NJ, DH + 1], BF, tag="vx1")
            bq = pp.tile([128, 1], F, tag="bq")
            bk = pp.tile([128, 1], F, tag="bk")
            ones32 = pp.tile([128, NJ], F, tag="ones32")
            warm = pp.tile([1, 1], F, tag="warm")
            wmm = pp.tile([64, 128], BF, tag="wmm")

            # pre-warm the exp table set while DMAs run
            nc.vector.memset(warm[:], 0.0)
            nc.scalar.activation(warm[:], warm[:], Act.Exp, scale=1.0)

            nc.vector.memset(wmm[:], 0.0)
            nc.vector.memset(ones32[:], 1.0)
            nc.vector.tensor_copy(vx0[:, :, DH:DH + 1], ones32[:, :, None])
            nc.vector.tensor_copy(vx1[:, :, DH:DH + 1], ones32[:, :, None])

            # Pools for the attention phase are opened before phase 1 is
            # emitted so the scheduler can overlap the phase-1 tail with
            # early score matmuls (PSUM: arena 5 + pv 2 + {ps1 2 during
            # phase1 / pso 1 after} = 8; chunk 0 only touches arena
            # slots 0-3).
            with tc.tile_pool(name="p2", bufs=1) as p2, \
                 tc.tile_pool(name="p2v", bufs=2) as p2v, \
                 tc.tile_pool(name="expp", bufs=20) as pe, \
                 tc.tile_pool(name="outp", bufs=4) as po, \
                 tc.tile_pool(name="ps2s", bufs=1, space="PSUM") as ps2s, \
                 tc.tile_pool(name="ps2v", bufs=1, space="PSUM") as ps2v:
                pss0 = ps2s.tile([128, 1024], F, tag="pss0")
                pss1 = ps2s.tile([128, 1024], F, tag="pss1")
                pv0 = ps2v.tile([128, 512], F, tag="pv0")
                pv1 = ps2v.tile([128, 512], F, tag="pv1")
                wo0 = p2.tile([DH, F_OUT], FR, tag="wo0")
                wo1 = p2.tile([DH, F_OUT], FR, tag="wo1")

                # ---- Phase 1: QKV projections over 4 quarters of L ----
                with tc.tile_pool(name="p1w", bufs=1) as p1w, \
                     tc.tile_pool(name="p1x", bufs=2) as p1x, \
                     tc.tile_pool(name="ps1", bufs=2, space="PSUM") as ps1:
                    wq = p1w.tile([128, KT, D2], BF, tag="wq")
                    wk = p1w.tile([128, KT, D2], BF, tag="wk")
                    wv = p1w.tile([128, KT, D2], BF, tag="wv")
                    ident = p1w.tile([128, 128], F, tag="ident")
                    for wt, wd in ((wk, wk_d), (wv, wv_d), (wq, wq_d)):
                        nc.scalar.dma_start(
                            out=wt[:],
                            in_=wd.ap().rearrange("(k p) d -> p k d", p=128),
                        )
                    make_identity(nc, ident[:])
                    nc.scalar.dma_start(out=bq[:], in_=bq_d.ap()[:, None])
                    nc.scalar.dma_start(out=bk[:], in_=bk_d.ap()[:, None])
                    nc.scalar.dma_start(out=wo0[:],
                                        in_=wo0_d.ap().bitcast(FR))
                    nc.scalar.dma_start(out=wo1[:],
                                        in_=wo1_d.ap().bitcast(FR))

                    # HAM warm-up: ~4us of dummy matmuls during the input
                    # DMA wait so the real projections run at 2.4 GHz
                    psw = ps1.tile([128, 512], F, tag="ps1")
                    for _ in range(80):
                        nc.tensor.matmul(
                            psw[0:64, 0:128], wmm[:, 0:64], wmm[:, :],
                            start=True, stop=True,
                        )

                    def proj(wt, dst, bias, xt, c0, g0):
                        ps = ps1.tile([128, 512], F, tag="ps1")
                        for kt in range(KT):
                            nc.tensor.matmul(
                                ps[:], wt[:, kt, :], xt[:, kt, c0:c0 + 512],
                                start=(kt == 0), stop=(kt == KT - 1),
                            )
                        if bias is not None:
                            nc.scalar.activation(
                                dst[:, g0:g0 + 512], ps[:], Act.Identity,
                                bias=bias[:], scale=1.0,
                            )
                        else:
                            nc.scalar.copy(dst[:, c0:c0 + 512], ps[:])

                    q_tiles = {}

                    def emit_q_dma(qq):
                        l0 = qq * QL
                        xt = p1x.tile([128, KT, QL], BF, tag="xt")
                        # per-kt pieces so each projection matmul can run
                        # as soon as its f-tile lands; halves split over
                        # two queues
                        for kt in range(KT):
                            nc.sync.dma_start(
                                out=xt[:, kt, 0:QL // 2],
                                in_=xT_d.ap()[kt * 128:(kt + 1) * 128,
                                              l0:l0 + QL // 2])
                        for kt in range(KT):
                            nc.gpsimd.dma_start(
                                out=xt[:, kt, QL // 2:QL],
                                in_=xT_d.ap()[kt * 128:(kt + 1) * 128,
                                              l0 + QL // 2:l0 + QL])
                        q_tiles[qq] = xt

                    def emit_q_comp(qq):
                        l0 = qq * QL
                        xt = q_tiles.pop(qq)
                        vTq = p1x.tile([128, QL], F, tag="vTq")
                        for ch in range(QL // 512):
                            proj(wk, kT, bk, xt, ch * 512, l0 + ch * 512)
                            proj(wv, vTq, None, xt, ch * 512, ch * 512)
                            if qq == 0:
                                proj(wq, qT, bq, xt, ch * 512, l0 + ch * 512)
                        for jl in range(QL // 128):
                            jt = qq * (QL // 128) + jl
                            pt = ps1.tile([128, 512], F, tag="ps1")
                            nc.tensor.transpose(
                                pt[:, 0:128],
                                vTq[:, jl * 128:(jl + 1) * 128], ident[:])
                            nc.vector.tensor_copy(vx0[:, jt, 0:DH], pt[:, 0:DH])
                            nc.vector.tensor_copy(vx1[:, jt, 0:DH],
                                                  pt[:, DH:D2])
                        if qq != 0:
                            for ch in range(QL // 512):
                                proj(wq, qT, bq, xt, ch * 512, l0 + ch * 512)

                    # chunk 0's attention interleaves into the remaining
                    # quarters so the in-order PE queue no longer serializes
                    # all of phase 1 ahead of the first scores matmul
                    emit_q_dma(0)
                    emit_q_comp(0)
                    emit_q_dma(1)
                    c0 = _phase2_chunk0(nc, (pss0, pss1), pv0, pv1, pe,
                                        p2v, qT, kT, vx0, vx1, NJ, F, FR,
                                        BF, I16, Act, Alu, emit_q_dma,
                                        emit_q_comp)

                if dbg:
                    nc.sync.dma_start(out=dbg_q.ap(), in_=qT[:].bitcast(F))
                    nc.sync.dma_start(out=dbg_k.ap(), in_=kT[:].bitcast(F))
                    nc.sync.dma_start(
                        out=dbg_v.ap(),
                        in_=vx0[:].bitcast(F).rearrange("p a b -> p (a b)"))

                # ---- Phase 2+3: attention, interleaved normalize/out-proj ----
                with tc.tile_pool(name="ps2o", bufs=2, space="PSUM") as ps2o:
                    _phase2(nc, (pss0, pss1), pv0, pv1, ps2o, pe, po, p2v,
                            qT, kT, vx0, vx1, wo0, wo1, out_d,
                            NI, NJ, F, FR, BF, I16, Act, Alu, c0)

    nc.compile()
    return nc


def _score_tile(tiles, n, NJ):
    return tiles[n % 2]


def _mk_emit_scores(nc, tiles, qT, kT, NJ):
    def emit_scores(n):
        ic, jt = n // NJ, n % NJ
        i0 = ic * 512
        j0 = jt * 128
        ps = _score_tile(tiles, n, NJ)
        nc.tensor.matmul(
            ps[:, 0:512], kT[0:64, j0:j0 + 128], qT[0:64, i0:i0 + 512],
            start=True, stop=True, tile_position=(0, 0),
        )
        nc.tensor.matmul(
            ps[:, 512:1024], kT[64:128, j0:j0 + 128],
            qT[64:128, i0:i0 + 512],
            start=True, stop=True, tile_position=(64, 0),
        )
    return emit_scores


def _mk_emit_exp(nc, tiles, NJ, BF, I16, Act, Alu):
    # Whole-tile exps alternating engines: per-op overhead (~0.2us pipe
    # drain + dispatch) makes one [128,1024] op per tile cheaper than
    # split halves, even though the halves would free the bank sooner.
    def emit_exp(n, jt, eT):
        ps = _score_tile(tiles, n, NJ)
        if jt % 2 == 1:
            nc.vector.tensor_scalar(
                eT[:].bitcast(I16), ps[:], SCH_A, SCH_B,
                Alu.mult, Alu.add)
        else:
            nc.scalar.activation(eT[:], ps[:], Act.Exp, scale=0.125)
    return emit_exp


def _mk_emit_chain(nc, pv0, pv1, vx0, vx1, NJ):
    """One 4-tile attn@v accumulation chain for one head.  Chains are
    spread ~evenly over the j-loop (the PE activity monitor re-throttles
    the clock if any ~3.4us window is mostly idle)."""
    def emit_chain(eTs, b0, head):
        pv, vx, c0 = ((pv0, vx0, 0) if head == 0 else (pv1, vx1, 512))
        for bjt in range(b0, b0 + 4):
            nc.tensor.matmul(
                pv[0:DH + 1, :], vx[:, bjt, :],
                eTs[bjt][:, c0:c0 + 512],
                start=(bjt == 0), stop=(bjt == NJ - 1),
            )
    return emit_chain


def _phase2(nc, stiles, pv0, pv1, ps2o, pe, po, p2v, qT, kT, vx0, vx1,
            wo0, wo1, out_d, NI, NJ, F, FR, BF, I16, Act, Alu, c0):
    emit_scores = _mk_emit_scores(nc, stiles, qT, kT, NJ)
    emit_exp = _mk_emit_exp(nc, stiles, NJ, BF, I16, Act, Alu)
    emit_chain = _mk_emit_chain(nc, pv0, pv1, vx0, vx1, NJ)

    def norm_unit(ic, p0, p1):
        # reciprocal + broadcast of the softmax denominators for chunk ic
        for (va, _), tg in ((p0, "0"), (p1, "1")):
            sh = p2v.tile([1, 512], F, tag="sh" + tg)
            rc = p2v.tile([1, 512], F, tag="rc" + tg)
            rb = p2v.tile([DH, 512], F, tag="rb" + tg)
            nc.sync.dma_start(out=sh[:], in_=va[DH:DH + 1, :].bitcast(F))
            nc.vector.reciprocal_approx_fast(out=rc[:], in_=sh[:])
            nc.gpsimd.partition_broadcast(rb[:], rc[:], channels=DH)
            nc.vector.tensor_mul(va[0:DH, :], va[0:DH, :], rb[:])

    def oproj_unit(ic, p0, p1, iw, fc, evac="s"):
        # one output-projection tile of chunk ic
        isl = slice(iw * 128, (iw + 1) * 128)
        r0 = ic * 512 + iw * 128
        f0 = fc * 512
        pso = ps2o.tile([128, 512], F, tag="pso")
        nc.tensor.matmul(
            pso[:], p0[0][0:DH, isl], wo0[:, f0:f0 + 512],
            start=True, stop=False,
        )
        nc.tensor.matmul(
            pso[:], p1[0][0:DH, isl], wo1[:, f0:f0 + 512],
            start=False, stop=True,
        )
        ot = po.tile([128, 512], F, tag="ot")
        if evac == "s":
            nc.scalar.copy(ot[:], pso[:])
            nc.sync.dma_start(
                out=out_d.ap()[r0:r0 + 128, f0:f0 + 512], in_=ot[:])
        else:
            nc.vector.tensor_copy(ot[:], pso[:])
            nc.gpsimd.dma_start(
                out=out_d.ap()[r0:r0 + 128, f0:f0 + 512], in_=ot[:])

    # attn@v chains at jts 5,6, 9,10, ..., 29,30 (b=(jt-5)//4, head par),
    # out-proj units of the previous chunk at the chain-free jts
    OPROJ_JTS = (7, 8, 11, 12, 15, 16, 19, 20)

    pending = c0["pending"]
    carry = c0["carry"]
    for ic in range(1, NI):
        units = []
        if pending is not None:
            pic, pp0, pp1 = pending[0], pending[1], pending[2]
            units = [(pic, pp0, pp1, iw, fc,
                      "v" if (iw * 2 + fc) % 3 == 2 else "s")
                     for iw in range(4) for fc in range(F_OUT // 512)]
        eTs = []
        for jt in range(NJ):
            n = ic * NJ + jt
            eT = pe.tile([128, 1024], BF, tag="eT")
            emit_exp(n, jt, eT)
            eTs.append(eT)
            if jt == 1 and carry is not None:
                carry(0)
            if jt == 2 and pending is not None:
                nc.vector.tensor_copy(pending[1][0][:], pv0[0:DH + 1, :])
            if jt == 3 and carry is not None:
                carry(1)
                carry = None
                nc.vector.tensor_copy(pending[2][0][:], pv1[0:DH + 1, :])
            if jt == 4 and pending is not None:
                norm_unit(pending[0], pending[1], pending[2])
            if jt >= 5 and (jt - 5) % 4 in (0, 1):
                emit_chain(eTs, 4 * ((jt - 5) // 4), (jt - 5) % 4)
            if jt in OPROJ_JTS and units:
                oproj_unit(*units.pop(0))
            if n + 1 < NI * NJ:
                emit_scores(n + 1)
        va0 = p2v.tile([DH + 1, 512], FR, tag="va0")
        va1 = p2v.tile([DH + 1, 512], FR, tag="va1")
        if ic < NI - 1:
            ceTs = eTs
            carry = (lambda head, e=ceTs: emit_chain(e, NJ - 4, head))
            pending = (ic, (va0, None), (va1, None))
        else:
            emit_chain(eTs, NJ - 4, 0)
            emit_chain(eTs, NJ - 4, 1)
            nc.vector.tensor_copy(va0[:], pv0[0:DH + 1, :])
            nc.vector.tensor_copy(va1[:], pv1[0:DH + 1, :])
            pending = (ic, (va0, None), (va1, None))

    norm_unit(pending[0], pending[1], pending[2])
    for iw in range(4):
        for fc in range(F_OUT // 512):
            oproj_unit(pending[0], pending[1], pending[2], iw, fc,
                       "s" if fc == 0 else "v")


def _phase2_chunk0(nc, stiles, pv0, pv1, pe, p2v, qT, kT, vx0, vx1, NJ,
                   F, FR, BF, I16, Act, Alu, emit_q_dma, emit_q_comp):
    """Chunk 0's j-loop, emitted inside the phase-1 pool scope with the
    remaining quarters' DMAs/compute interleaved at fixed j-slots."""
    emit_scores = _mk_emit_scores(nc, stiles, qT, kT, NJ)
    emit_exp = _mk_emit_exp(nc, stiles, NJ, BF, I16, Act, Alu)
    emit_chain = _mk_emit_chain(nc, pv0, pv1, vx0, vx1, NJ)

    eTs = []
    emit_scores(0)
    for jt in range(NJ):
        eT = pe.tile([128, 1024], BF, tag="eT")
        emit_exp(jt, jt, eT)
        eTs.append(eT)
        if jt >= 5 and (jt - 5) % 4 in (0, 1):
            emit_chain(eTs, 4 * ((jt - 5) // 4), (jt - 5) % 4)
        if jt % 8 == 6 and jt < NJ - 2:
            emit_q_comp(jt // 8 + 1)
            if jt // 8 + 2 < 4:
                emit_q_dma(jt // 8 + 2)
        emit_scores(jt + 1)
    va0 = p2v.tile([DH + 1, 512], FR, tag="va0")
    va1 = p2v.tile([DH + 1, 512], FR, tag="va1")
    carry = (lambda head, e=eTs: emit_chain(e, NJ - 4, head))
    return {"pending": (0, (va0, None), (va1, None)), "carry": carry}


def _get_built():
    global _BUILT
    if _BUILT is None:
        _BUILT = _build()
    return _BUILT


def kernel(x, Wq, bq, Wk, bk, Wv, bv, Wo, bo):
    from concourse.bass_utils import run_bass_kernel_spmd

    x = np.ascontiguousarray(np.asarray(x, dtype=np.float32))
    Wq = np.asarray(Wq, dtype=np.float32)
    Wk = np.asarray(Wk, dtype=np.float32)
    Wv = np.asarray(Wv, dtype=np.float32)
    Wo = np.asarray(Wo, dtype=np.float32)
    bq = np.asarray(bq, dtype=np.float32)
    bk = np.asarray(bk, dtype=np.float32)
    bv = np.asarray(bv, dtype=np.float32)
    bo = np.asarray(bo, dtype=np.float32)

    nc = _get_built()

    import ml_dtypes
    BFH = ml_dtypes.bfloat16
    xT = np.ascontiguousarray(x.T.astype(BFH))  # [F_IN, L] bf16
    in_maps = []
    for c in range(NCORES):
        hs = slice(c * HPC, (c + 1) * HPC)
        in_maps.append({
            "xT": xT,
            "wq": np.ascontiguousarray(
                Wq[:, hs, :].reshape(F_IN, D2).astype(BFH)),
            "wk": np.ascontiguousarray(
                Wk[:, hs, :].reshape(F_IN, D2).astype(BFH)),
            "wv": np.ascontiguousarray(
                Wv[:, hs, :].reshape(F_IN, D2).astype(BFH)),
            "bq": np.ascontiguousarray(bq[hs].reshape(D2)),
            "bk": np.ascontiguousarray(bk[hs].reshape(D2)),
            "wo0": np.ascontiguousarray(Wo[c * HPC]),
            "wo1": np.ascontiguousarray(Wo[c * HPC + 1]),
        })

    res = run_bass_kernel_spmd(nc, in_maps, list(range(NCORES)))
    acc = np.zeros((L, F_OUT), dtype=np.float64)
    for c in range(NCORES):
        acc += res.results[c]["out"].astype(np.float64)
    # bv contribution (softmax rows sum to 1) + bo, both exact on host
    acc += (bv.reshape(1, H * DH).astype(np.float64)
            @ Wo.reshape(H * DH, F_OUT).astype(np.float64))
    acc += bo.astype(np.float64)
    return acc.astype(np.float32)


# revision 26
# speedup vs baseline: 1.1069x; 1.0102x over previous
"""Multi-head self-attention Trainium2 kernel (8 NeuronCores, head-parallel).

Problem: L=4096, F_IN=1024, H=16, DH=64, F_OUT=1024, fp32.
Sharding: 2 heads per core (tensor parallel over heads). Each core computes
its 2 heads' attention and its partial output projection; the host sums the
8 partials (the all-reduce of the sharding hint, done at gather time).

Numerics: x and Wq/Wk/Wv are loaded in bf16; projections accumulate in fp32
PSUM. The attention matmuls run in bf16 with fp32 PSUM accumulation. Exps
alternate between ScalarE (exact ACT exp) and VectorE (Schraudolph
int16-bitcast approximation); the softmax denominator is summed from the
*rounded* attention weights (ones-column trick), so rounding largely
cancels in the normalization.

Schedule design (from perfetto analysis):
  - The binding constraint is a latency loop: scores(n+k) cannot issue
    until the PSUM bank of scores(n) is freed by exp(n). Phase 2 uses a
    persistent 5-bank scores arena with slots rotating mod 5 (2.5 tiles in
    flight) so the loop latency (scores + sem + exp + sem ~ 1.6us) divides
    by 2.5 and the PE becomes the pacer. Chunk 0 uses a 4-slot aligned
    rotation because phase 1 still holds 2 PSUM banks.
  - Per-jt emission order: exp first, then long-ready PE work (attn@v
    lagged 16 tiles, out-proj of the previous chunk), and the bank-gated
    scores LAST so a stall cannot head-of-line-block ready matmuls.
  - Chunk-end work (final attn@v block, pv evac, norm) is deferred into
    the next chunk's first iterations.
  - Input DMAs are spread across HW queues (sync/vector/gpsimd) and the
    x quarters are fetched half-first so the first k-projection can start
    as early as possible; a burst of dummy matmuls warms the PE HAM clock
    gate during the initial DMA wait.
"""

import numpy as np

L, F_IN, H, DH, F_OUT = 4096, 1024, 16, 64, 1024

# Schraudolph exp constants (DVE): int16(ps*SCH_A + SCH_B) bitcast bf16
SCH_C = 0.0579
SCH_A = 128.0 * 1.4426950408889634 * 0.125
SCH_B = 128.0 * (127.0 - SCH_C)
NCORES = 8
HPC = H // NCORES  # heads per core = 2
D2 = HPC * DH      # 128, per-core packed head dim

_BUILT = None


def _build():
    import os

    import concourse.bass as bass  # noqa: F401
    import concourse.mybir as mybir
    import concourse.tile as tile
    from concourse import bacc
    from concourse.masks import make_identity

    F = mybir.dt.float32
    FR = mybir.dt.float32r
    BF = mybir.dt.bfloat16
    I16 = mybir.dt.int16
    Alu = mybir.AluOpType
    Act = mybir.ActivationFunctionType

    nc = bacc.Bacc("TRN2", target_bir_lowering=False, debug=False)

    xT_d = nc.declare_dram_parameter("xT", [F_IN, L], BF, isOutput=False)
    wq_d = nc.declare_dram_parameter("wq", [F_IN, D2], BF, isOutput=False)
    wk_d = nc.declare_dram_parameter("wk", [F_IN, D2], BF, isOutput=False)
    wv_d = nc.declare_dram_parameter("wv", [F_IN, D2], BF, isOutput=False)
    bq_d = nc.declare_dram_parameter("bq", [D2], F, isOutput=False)
    bk_d = nc.declare_dram_parameter("bk", [D2], F, isOutput=False)
    wo0_d = nc.declare_dram_parameter("wo0", [DH, F_OUT], F, isOutput=False)
    wo1_d = nc.declare_dram_parameter("wo1", [DH, F_OUT], F, isOutput=False)
    out_d = nc.declare_dram_parameter("out", [L, F_OUT], F, isOutput=True)

    dbg = bool(os.environ.get("K_DEBUG"))
    if dbg:
        dbg_q = nc.declare_dram_parameter("dbg_q", [128, L], F, isOutput=True)
        dbg_k = nc.declare_dram_parameter("dbg_k", [128, L], F, isOutput=True)
        dbg_v = nc.declare_dram_parameter("dbg_v", [128, 32 * 65], F, isOutput=True)

    KT = F_IN // 128   # 8 f-tiles
    NI = L // 512      # 8 i-chunks
    NJ = L // 128      # 32 j-tiles
    QL = 1024          # quarter width in L
    NQ = L // QL       # 4 quarters

    with tile.TileContext(nc) as tc:
        with tc.tile_pool(name="persist", bufs=1) as pp:
            qT = pp.tile([128, L], BF, tag="qT")             # [d2, i]
            kT = pp.tile([128, L], BF, tag="kT")             # [d2, j]
            vx0 = pp.tile([128, NJ, DH + 1], BF, tag="vx0")  # [j_in, jt, d|1]
            vx1 = pp.tile([128, NJ, DH + 1], BF, tag="vx1")
            bq = pp.tile([128, 1], F, tag="bq")
            bk = pp.tile([128, 1], F, tag="bk")
            ones32 = pp.tile([128, NJ], F, tag="ones32")
            warm = pp.tile([1, 1], F, tag="warm")
            wmm = pp.tile([64, 128], BF, tag="wmm")

            # pre-warm the exp table set while DMAs run
            nc.vector.memset(warm[:], 0.0)
            nc.scalar.activation(warm[:], warm[:], Act.Exp, scale=1.0)

            nc.vector.memset(wmm[:], 0.0)
            nc.vector.memset(ones32[:], 1.0)
            nc.vector.tensor_copy(vx0[:, :, DH:DH + 1], ones32[:, :, None])
            nc.vector.tensor_copy(vx1[:, :, DH:DH + 1], ones32[:, :, None])

            # Pools for the attention phase are opened before phase 1 is
            # emitted so the scheduler can overlap the phase-1 tail with
            # early score matmuls (PSUM: arena 5 + pv 2 + {ps1 2 during
            # phase1 / pso 1 after} = 8; chunk 0 only touches arena
            # slots 0-3).
            with tc.tile_pool(name="p2", bufs=1) as p2, \
                 tc.tile_pool(name="p2v", bufs=2) as p2v, \
                 tc.tile_pool(name="expp", bufs=20) as pe, \
                 tc.tile_pool(name="outp", bufs=4) as po, \
                 tc.tile_pool(name="ps2s", bufs=1, space="PSUM") as ps2s, \
                 tc.tile_pool(name="ps2v", bufs=1, space="PSUM") as ps2v:
                pss0 = ps2s.tile([128, 1024], F, tag="pss0")
                pss1 = ps2s.tile([128, 1024], F, tag="pss1")
                pv0 = ps2v.tile([128, 512], F, tag="pv0")
                pv1 = ps2v.tile([128, 512], F, tag="pv1")
                wo0 = p2.tile([DH, F_OUT], FR, tag="wo0")
                wo1 = p2.tile([DH, F_OUT], FR, tag="wo1")

                # ---- Phase 1: QKV projections over 4 quarters of L ----
                with tc.tile_pool(name="p1w", bufs=1) as p1w, \
                     tc.tile_pool(name="p1x", bufs=2) as p1x, \
                     tc.tile_pool(name="ps1", bufs=2, space="PSUM") as ps1:
                    wq = p1w.tile([128, KT, D2], BF, tag="wq")
                    wk = p1w.tile([128, KT, D2], BF, tag="wk")
                    wv = p1w.tile([128, KT, D2], BF, tag="wv")
                    ident = p1w.tile([128, 128], F, tag="ident")
                    for wt, wd in ((wk, wk_d), (wv, wv_d), (wq, wq_d)):
                        nc.scalar.dma_start(
                            out=wt[:],
                            in_=wd.ap().rearrange("(k p) d -> p k d", p=128),
                        )
                    make_identity(nc, ident[:])
                    nc.scalar.dma_start(out=bq[:], in_=bq_d.ap()[:, None])
                    nc.scalar.dma_start(out=bk[:], in_=bk_d.ap()[:, None])
                    nc.scalar.dma_start(out=wo0[:],
                                        in_=wo0_d.ap().bitcast(FR))
                    nc.scalar.dma_start(out=wo1[:],
                                        in_=wo1_d.ap().bitcast(FR))

                    # HAM warm-up: ~4us of dummy matmuls during the input
                    # DMA wait so the real projections run at 2.4 GHz
                    psw = ps1.tile([128, 512], F, tag="ps1")
                    for _ in range(54):
                        nc.tensor.matmul(
                            psw[0:64, 0:128], wmm[:, 0:64], wmm[:, :],
                            start=True, stop=True,
                        )

                    def proj(wt, dst, bias, xt, c0, g0):
                        ps = ps1.tile([128, 512], F, tag="ps1")
                        for kt in range(KT):
                            nc.tensor.matmul(
                                ps[:], wt[:, kt, :], xt[:, kt, c0:c0 + 512],
                                start=(kt == 0), stop=(kt == KT - 1),
                            )
                        if bias is not None:
                            nc.scalar.activation(
                                dst[:, g0:g0 + 512], ps[:], Act.Identity,
                                bias=bias[:], scale=1.0,
                            )
                        else:
                            nc.scalar.copy(dst[:, c0:c0 + 512], ps[:])

                    q_tiles = {}

                    def emit_q_dma(qq):
                        l0 = qq * QL
                        xt = p1x.tile([128, KT, QL], BF, tag="xt")
                        # per-kt pieces so each projection matmul can run
                        # as soon as its f-tile lands; halves split over
                        # two queues
                        for kt in range(KT):
                            nc.sync.dma_start(
                                out=xt[:, kt, 0:QL // 2],
                                in_=xT_d.ap()[kt * 128:(kt + 1) * 128,
                                              l0:l0 + QL // 2])
                        for kt in range(KT):
                            nc.gpsimd.dma_start(
                                out=xt[:, kt, QL // 2:QL],
                                in_=xT_d.ap()[kt * 128:(kt + 1) * 128,
                                              l0 + QL // 2:l0 + QL])
                        q_tiles[qq] = xt

                    def emit_q_comp(qq):
                        l0 = qq * QL
                        xt = q_tiles.pop(qq)
                        vTq = p1x.tile([128, QL], F, tag="vTq")
                        for ch in range(QL // 512):
                            proj(wk, kT, bk, xt, ch * 512, l0 + ch * 512)
                            proj(wv, vTq, None, xt, ch * 512, ch * 512)
                            if qq == 0:
                                proj(wq, qT, bq, xt, ch * 512, l0 + ch * 512)
                        for jl in range(QL // 128):
                            jt = qq * (QL // 128) + jl
                            pt = ps1.tile([128, 512], F, tag="ps1")
                            nc.tensor.transpose(
                                pt[:, 0:128],
                                vTq[:, jl * 128:(jl + 1) * 128], ident[:])
                            nc.vector.tensor_copy(vx0[:, jt, 0:DH], pt[:, 0:DH])
                            nc.vector.tensor_copy(vx1[:, jt, 0:DH],
                                                  pt[:, DH:D2])
                        if qq != 0:
                            for ch in range(QL // 512):
                                proj(wq, qT, bq, xt, ch * 512, l0 + ch * 512)

                    # chunk 0's attention interleaves into the remaining
                    # quarters so the in-order PE queue no longer serializes
                    # all of phase 1 ahead of the first scores matmul
                    emit_q_dma(0)
                    emit_q_comp(0)
                    emit_q_dma(1)
                    c0 = _phase2_chunk0(nc, (pss0, pss1), pv0, pv1, pe,
                                        p2v, qT, kT, vx0, vx1, NJ, F, FR,
                                        BF, I16, Act, Alu, emit_q_dma,
                                        emit_q_comp)

                if dbg:
                    nc.sync.dma_start(out=dbg_q.ap(), in_=qT[:].bitcast(F))
                    nc.sync.dma_start(out=dbg_k.ap(), in_=kT[:].bitcast(F))
                    nc.sync.dma_start(
                        out=dbg_v.ap(),
                        in_=vx0[:].bitcast(F).rearrange("p a b -> p (a b)"))

                # ---- Phase 2+3: attention, interleaved normalize/out-proj ----
                with tc.tile_pool(name="ps2o", bufs=2, space="PSUM") as ps2o:
                    _phase2(nc, (pss0, pss1), pv0, pv1, ps2o, pe, po, p2v,
                            qT, kT, vx0, vx1, wo0, wo1, out_d,
                            NI, NJ, F, FR, BF, I16, Act, Alu, c0)

    nc.compile()
    return nc


def _score_tile(tiles, n, NJ):
    return tiles[n % 2]


def _mk_emit_scores(nc, tiles, qT, kT, NJ):
    def emit_scores(n):
        ic, jt = n // NJ, n % NJ
        i0 = ic * 512
        j0 = jt * 128
        ps = _score_tile(tiles, n, NJ)
        nc.tensor.matmul(
            ps[:, 0:512], kT[0:64, j0:j0 + 128], qT[0:64, i0:i0 + 512],
            start=True, stop=True, tile_position=(0, 0),
        )
        nc.tensor.matmul(
            ps[:, 512:1024], kT[64:128, j0:j0 + 128],
            qT[64:128, i0:i0 + 512],
            start=True, stop=True, tile_position=(64, 0),
        )
    return emit_scores


def _mk_emit_exp(nc, tiles, NJ, BF, I16, Act, Alu):
    # Whole-tile exps alternating engines: per-op overhead (~0.2us pipe
    # drain + dispatch) makes one [128,1024] op per tile cheaper than
    # split halves, even though the halves would free the bank sooner.
    def emit_exp(n, jt, eT):
        ps = _score_tile(tiles, n, NJ)
        if jt % 2 == 1:
            nc.vector.tensor_scalar(
                eT[:].bitcast(I16), ps[:], SCH_A, SCH_B,
                Alu.mult, Alu.add)
        else:
            nc.scalar.activation(eT[:], ps[:], Act.Exp, scale=0.125)
    return emit_exp


def _mk_emit_chain(nc, pv0, pv1, vx0, vx1, NJ):
    """One 4-tile attn@v accumulation chain for one head.  Chains are
    spread ~evenly over the j-loop (the PE activity monitor re-throttles
    the clock if any ~3.4us window is mostly idle)."""
    def emit_chain(eTs, b0, head):
        pv, vx, c0 = ((pv0, vx0, 0) if head == 0 else (pv1, vx1, 512))
        for bjt in range(b0, b0 + 4):
            nc.tensor.matmul(
                pv[0:DH + 1, :], vx[:, bjt, :],
                eTs[bjt][:, c0:c0 + 512],
                start=(bjt == 0), stop=(bjt == NJ - 1),
            )
    return emit_chain


def _phase2(nc, stiles, pv0, pv1, ps2o, pe, po, p2v, qT, kT, vx0, vx1,
            wo0, wo1, out_d, NI, NJ, F, FR, BF, I16, Act, Alu, c0):
    emit_scores = _mk_emit_scores(nc, stiles, qT, kT, NJ)
    emit_exp = _mk_emit_exp(nc, stiles, NJ, BF, I16, Act, Alu)
    emit_chain = _mk_emit_chain(nc, pv0, pv1, vx0, vx1, NJ)

    def norm_unit(ic, p0, p1):
        # reciprocal + broadcast of the softmax denominators for chunk ic
        for (va, _), tg in ((p0, "0"), (p1, "1")):
            sh = p2v.tile([1, 512], F, tag="sh" + tg)
            rc = p2v.tile([1, 512], F, tag="rc" + tg)
            rb = p2v.tile([DH, 512], F, tag="rb" + tg)
            nc.sync.dma_start(out=sh[:], in_=va[DH:DH + 1, :].bitcast(F))
            nc.vector.reciprocal_approx_fast(out=rc[:], in_=sh[:])
            nc.gpsimd.partition_broadcast(rb[:], rc[:], channels=DH)
            nc.vector.tensor_mul(va[0:DH, :], va[0:DH, :], rb[:])

    def oproj_unit(ic, p0, p1, iw, fc, evac="s"):
        # one output-projection tile of chunk ic
        isl = slice(iw * 128, (iw + 1) * 128)
        r0 = ic * 512 + iw * 128
        f0 = fc * 512
        pso = ps2o.tile([128, 512], F, tag="pso")
        nc.tensor.matmul(
            pso[:], p0[0][0:DH, isl], wo0[:, f0:f0 + 512],
            start=True, stop=False,
        )
        nc.tensor.matmul(
            pso[:], p1[0][0:DH, isl], wo1[:, f0:f0 + 512],
            start=False, stop=True,
        )
        ot = po.tile([128, 512], F, tag="ot")
        if evac == "s":
            nc.scalar.copy(ot[:], pso[:])
        else:
            nc.vector.tensor_copy(ot[:], pso[:])
        nc.sync.dma_start(
            out=out_d.ap()[r0:r0 + 128, f0:f0 + 512], in_=ot[:])

    # attn@v chains at jts 5,6, 9,10, ..., 29,30 (b=(jt-5)//4, head par),
    # out-proj units of the previous chunk at the chain-free jts
    OPROJ_JTS = (7, 8, 11, 12, 15, 16, 19, 20)

    pending = c0["pending"]
    carry = c0["carry"]
    for ic in range(1, NI):
        units = []
        if pending is not None:
            pic, pp0, pp1 = pending[0], pending[1], pending[2]
            units = [(pic, pp0, pp1, iw, fc,
                      "v" if (iw * 2 + fc) % 3 == 2 else "s")
                     for iw in range(4) for fc in range(F_OUT // 512)]
        eTs = []
        for jt in range(NJ):
            n = ic * NJ + jt
            eT = pe.tile([128, 1024], BF, tag="eT")
            emit_exp(n, jt, eT)
            eTs.append(eT)
            if jt == 1 and carry is not None:
                carry(0)
            if jt == 2 and pending is not None:
                nc.vector.tensor_copy(pending[1][0][:], pv0[0:DH + 1, :])
            if jt == 3 and carry is not None:
                carry(1)
                carry = None
                nc.vector.tensor_copy(pending[2][0][:], pv1[0:DH + 1, :])
            if jt == 4 and pending is not None:
                norm_unit(pending[0], pending[1], pending[2])
            if jt >= 5 and (jt - 5) % 4 in (0, 1):
                emit_chain(eTs, 4 * ((jt - 5) // 4), (jt - 5) % 4)
            if jt in OPROJ_JTS and units:
                oproj_unit(*units.pop(0))
            if n + 1 < NI * NJ:
                emit_scores(n + 1)
        va0 = p2v.tile([DH + 1, 512], FR, tag="va0")
        va1 = p2v.tile([DH + 1, 512], FR, tag="va1")
        if ic < NI - 1:
            ceTs = eTs
            carry = (lambda head, e=ceTs: emit_chain(e, NJ - 4, head))
            pending = (ic, (va0, None), (va1, None))
        else:
            emit_chain(eTs, NJ - 4, 0)
            emit_chain(eTs, NJ - 4, 1)
            nc.vector.tensor_copy(va0[:], pv0[0:DH + 1, :])
            nc.vector.tensor_copy(va1[:], pv1[0:DH + 1, :])
            pending = (ic, (va0, None), (va1, None))

    norm_unit(pending[0], pending[1], pending[2])
    for iw in range(4):
        for fc in range(F_OUT // 512):
            oproj_unit(pending[0], pending[1], pending[2], iw, fc,
                       "s" if fc == 0 else "v")


def _phase2_chunk0(nc, stiles, pv0, pv1, pe, p2v, qT, kT, vx0, vx1, NJ,
                   F, FR, BF, I16, Act, Alu, emit_q_dma, emit_q_comp):
    """Chunk 0's j-loop, emitted inside the phase-1 pool scope with the
    remaining quarters' DMAs/compute interleaved at fixed j-slots."""
    emit_scores = _mk_emit_scores(nc, stiles, qT, kT, NJ)
    emit_exp = _mk_emit_exp(nc, stiles, NJ, BF, I16, Act, Alu)
    emit_chain = _mk_emit_chain(nc, pv0, pv1, vx0, vx1, NJ)

    eTs = []
    emit_scores(0)
    for jt in range(NJ):
        eT = pe.tile([128, 1024], BF, tag="eT")
        emit_exp(jt, jt, eT)
        eTs.append(eT)
        if jt >= 5 and (jt - 5) % 4 in (0, 1):
            emit_chain(eTs, 4 * ((jt - 5) // 4), (jt - 5) % 4)
        if jt % 8 == 6 and jt < NJ - 2:
            emit_q_comp(jt // 8 + 1)
            if jt // 8 + 2 < 4:
                emit_q_dma(jt // 8 + 2)
        emit_scores(jt + 1)
    va0 = p2v.tile([DH + 1, 512], FR, tag="va0")
    va1 = p2v.tile([DH + 1, 512], FR, tag="va1")
    carry = (lambda head, e=eTs: emit_chain(e, NJ - 4, head))
    return {"pending": (0, (va0, None), (va1, None)), "carry": carry}


def _get_built():
    global _BUILT
    if _BUILT is None:
        _BUILT = _build()
    return _BUILT


def kernel(x, Wq, bq, Wk, bk, Wv, bv, Wo, bo):
    from concourse.bass_utils import run_bass_kernel_spmd

    x = np.ascontiguousarray(np.asarray(x, dtype=np.float32))
    Wq = np.asarray(Wq, dtype=np.float32)
    Wk = np.asarray(Wk, dtype=np.float32)
    Wv = np.asarray(Wv, dtype=np.float32)
    Wo = np.asarray(Wo, dtype=np.float32)
    bq = np.asarray(bq, dtype=np.float32)
    bk = np.asarray(bk, dtype=np.float32)
    bv = np.asarray(bv, dtype=np.float32)
    bo = np.asarray(bo, dtype=np.float32)

    nc = _get_built()

    import ml_dtypes
    BFH = ml_dtypes.bfloat16
    xT = np.ascontiguousarray(x.T.astype(BFH))  # [F_IN, L] bf16
    in_maps = []
    for c in range(NCORES):
        hs = slice(c * HPC, (c + 1) * HPC)
        in_maps.append({
            "xT": xT,
            "wq": np.ascontiguousarray(
                Wq[:, hs, :].reshape(F_IN, D2).astype(BFH)),
            "wk": np.ascontiguousarray(
                Wk[:, hs, :].reshape(F_IN, D2).astype(BFH)),
            "wv": np.ascontiguousarray(
                Wv[:, hs, :].reshape(F_IN, D2).astype(BFH)),
            "bq": np.ascontiguousarray(bq[hs].reshape(D2)),
            "bk": np.ascontiguousarray(bk[hs].reshape(D2)),
            "wo0": np.ascontiguousarray(Wo[c * HPC]),
            "wo1": np.ascontiguousarray(Wo[c * HPC + 1]),
        })

    res = run_bass_kernel_spmd(nc, in_maps, list(range(NCORES)))
    acc = np.zeros((L, F_OUT), dtype=np.float64)
    for c in range(NCORES):
        acc += res.results[c]["out"].astype(np.float64)
    # bv contribution (softmax rows sum to 1) + bo, both exact on host
    acc += (bv.reshape(1, H * DH).astype(np.float64)
            @ Wo.reshape(H * DH, F_OUT).astype(np.float64))
    acc += bo.astype(np.float64)
    return acc.astype(np.float32)


# revision 27
# speedup vs baseline: 1.1195x; 1.0114x over previous
"""Multi-head self-attention Trainium2 kernel (8 NeuronCores, head-parallel).

Problem: L=4096, F_IN=1024, H=16, DH=64, F_OUT=1024, fp32.
Sharding: 2 heads per core (tensor parallel over heads). Each core computes
its 2 heads' attention and its partial output projection; the host sums the
8 partials (the all-reduce of the sharding hint, done at gather time).

Numerics: x and Wq/Wk/Wv are loaded in bf16; projections accumulate in fp32
PSUM. The attention matmuls run in bf16 with fp32 PSUM accumulation. Exps
alternate between ScalarE (exact ACT exp) and VectorE (Schraudolph
int16-bitcast approximation); the softmax denominator is summed from the
*rounded* attention weights (ones-column trick), so rounding largely
cancels in the normalization.

Schedule design (from perfetto analysis):
  - The binding constraint is a latency loop: scores(n+k) cannot issue
    until the PSUM bank of scores(n) is freed by exp(n). Phase 2 uses a
    persistent 5-bank scores arena with slots rotating mod 5 (2.5 tiles in
    flight) so the loop latency (scores + sem + exp + sem ~ 1.6us) divides
    by 2.5 and the PE becomes the pacer. Chunk 0 uses a 4-slot aligned
    rotation because phase 1 still holds 2 PSUM banks.
  - Per-jt emission order: exp first, then long-ready PE work (attn@v
    lagged 16 tiles, out-proj of the previous chunk), and the bank-gated
    scores LAST so a stall cannot head-of-line-block ready matmuls.
  - Chunk-end work (final attn@v block, pv evac, norm) is deferred into
    the next chunk's first iterations.
  - Input DMAs are spread across HW queues (sync/vector/gpsimd) and the
    x quarters are fetched half-first so the first k-projection can start
    as early as possible; a burst of dummy matmuls warms the PE HAM clock
    gate during the initial DMA wait.
"""

import numpy as np

L, F_IN, H, DH, F_OUT = 4096, 1024, 16, 64, 1024

# Schraudolph exp constants (DVE): int16(ps*SCH_A + SCH_B) bitcast bf16
SCH_C = 0.0579
SCH_A = 128.0 * 1.4426950408889634 * 0.125
SCH_B = 128.0 * (127.0 - SCH_C)
NCORES = 8
HPC = H // NCORES  # heads per core = 2
D2 = HPC * DH      # 128, per-core packed head dim

_BUILT = None


def _build():
    import os

    import concourse.bass as bass  # noqa: F401
    import concourse.mybir as mybir
    import concourse.tile as tile
    from concourse import bacc
    from concourse.masks import make_identity

    F = mybir.dt.float32
    FR = mybir.dt.float32r
    BF = mybir.dt.bfloat16
    I16 = mybir.dt.int16
    Alu = mybir.AluOpType
    Act = mybir.ActivationFunctionType

    nc = bacc.Bacc("TRN2", target_bir_lowering=False, debug=False)

    xT_d = nc.declare_dram_parameter("xT", [F_IN, L], BF, isOutput=False)
    wq_d = nc.declare_dram_parameter("wq", [F_IN, D2], BF, isOutput=False)
    wk_d = nc.declare_dram_parameter("wk", [F_IN, D2], BF, isOutput=False)
    wv_d = nc.declare_dram_parameter("wv", [F_IN, D2], BF, isOutput=False)
    bq_d = nc.declare_dram_parameter("bq", [D2], F, isOutput=False)
    bk_d = nc.declare_dram_parameter("bk", [D2], F, isOutput=False)
    wo0_d = nc.declare_dram_parameter("wo0", [DH, F_OUT], F, isOutput=False)
    wo1_d = nc.declare_dram_parameter("wo1", [DH, F_OUT], F, isOutput=False)
    out_d = nc.declare_dram_parameter("out", [L, F_OUT], F, isOutput=True)

    dbg = bool(os.environ.get("K_DEBUG"))
    if dbg:
        dbg_q = nc.declare_dram_parameter("dbg_q", [128, L], F, isOutput=True)
        dbg_k = nc.declare_dram_parameter("dbg_k", [128, L], F, isOutput=True)
        dbg_v = nc.declare_dram_parameter("dbg_v", [128, 32 * 65], F, isOutput=True)

    KT = F_IN // 128   # 8 f-tiles
    NI = L // 512      # 8 i-chunks
    NJ = L // 128      # 32 j-tiles
    QL = 1024          # quarter width in L
    NQ = L // QL       # 4 quarters

    with tile.TileContext(nc) as tc:
        with tc.tile_pool(name="persist", bufs=1) as pp:
            qT = pp.tile([128, L], BF, tag="qT")             # [d2, i]
            kT = pp.tile([128, L], BF, tag="kT")             # [d2, j]
            vx0 = pp.tile([128, NJ, DH + 1], BF, tag="vx0")  # [j_in, jt, d|1]
            vx1 = pp.tile([128, NJ, DH + 1], BF, tag="vx1")
            bq = pp.tile([128, 1], F, tag="bq")
            bk = pp.tile([128, 1], F, tag="bk")
            ones32 = pp.tile([128, NJ], F, tag="ones32")
            warm = pp.tile([1, 1], F, tag="warm")
            wmm = pp.tile([64, 128], BF, tag="wmm")

            # pre-warm the exp table set while DMAs run
            nc.vector.memset(warm[:], 0.0)
            nc.scalar.activation(warm[:], warm[:], Act.Exp, scale=1.0)

            nc.vector.memset(wmm[:], 0.0)
            nc.vector.memset(ones32[:], 1.0)
            nc.vector.tensor_copy(vx0[:, :, DH:DH + 1], ones32[:, :, None])
            nc.vector.tensor_copy(vx1[:, :, DH:DH + 1], ones32[:, :, None])

            # Pools for the attention phase are opened before phase 1 is
            # emitted so the scheduler can overlap the phase-1 tail with
            # early score matmuls (PSUM: arena 5 + pv 2 + {ps1 2 during
            # phase1 / pso 1 after} = 8; chunk 0 only touches arena
            # slots 0-3).
            with tc.tile_pool(name="p2", bufs=1) as p2, \
                 tc.tile_pool(name="p2v", bufs=2) as p2v, \
                 tc.tile_pool(name="expp", bufs=20) as pe, \
                 tc.tile_pool(name="outp", bufs=4) as po, \
                 tc.tile_pool(name="ps2s", bufs=1, space="PSUM") as ps2s, \
                 tc.tile_pool(name="ps2v", bufs=1, space="PSUM") as ps2v:
                pss0 = ps2s.tile([128, 1024], F, tag="pss0")
                pss1 = ps2s.tile([128, 1024], F, tag="pss1")
                pv0 = ps2v.tile([128, 512], F, tag="pv0")
                pv1 = ps2v.tile([128, 512], F, tag="pv1")
                wo0 = p2.tile([DH, F_OUT], FR, tag="wo0")
                wo1 = p2.tile([DH, F_OUT], FR, tag="wo1")

                # ---- Phase 1: QKV projections over 4 quarters of L ----
                with tc.tile_pool(name="p1w", bufs=1) as p1w, \
                     tc.tile_pool(name="p1x", bufs=2) as p1x, \
                     tc.tile_pool(name="ps1", bufs=2, space="PSUM") as ps1:
                    wq = p1w.tile([128, KT, D2], BF, tag="wq")
                    wk = p1w.tile([128, KT, D2], BF, tag="wk")
                    wv = p1w.tile([128, KT, D2], BF, tag="wv")
                    ident = p1w.tile([128, 128], F, tag="ident")
                    for wt, wd in ((wk, wk_d), (wv, wv_d), (wq, wq_d)):
                        nc.scalar.dma_start(
                            out=wt[:],
                            in_=wd.ap().rearrange("(k p) d -> p k d", p=128),
                        )
                    make_identity(nc, ident[:])
                    nc.scalar.dma_start(out=bq[:], in_=bq_d.ap()[:, None])
                    nc.scalar.dma_start(out=bk[:], in_=bk_d.ap()[:, None])
                    nc.scalar.dma_start(out=wo0[:],
                                        in_=wo0_d.ap().bitcast(FR))
                    nc.scalar.dma_start(out=wo1[:],
                                        in_=wo1_d.ap().bitcast(FR))

                    # HAM warm-up: ~4us of dummy matmuls during the input
                    # DMA wait so the real projections run at 2.4 GHz
                    psw = ps1.tile([128, 512], F, tag="ps1")
                    for _ in range(36):
                        nc.tensor.matmul(
                            psw[0:64, 0:128], wmm[:, 0:64], wmm[:, :],
                            start=True, stop=True,
                        )

                    def proj(wt, dst, bias, xt, c0, g0):
                        ps = ps1.tile([128, 512], F, tag="ps1")
                        for kt in range(KT):
                            nc.tensor.matmul(
                                ps[:], wt[:, kt, :], xt[:, kt, c0:c0 + 512],
                                start=(kt == 0), stop=(kt == KT - 1),
                            )
                        if bias is not None:
                            nc.scalar.activation(
                                dst[:, g0:g0 + 512], ps[:], Act.Identity,
                                bias=bias[:], scale=1.0,
                            )
                        else:
                            nc.scalar.copy(dst[:, c0:c0 + 512], ps[:])

                    q_tiles = {}

                    xTr = xT_d.ap().rearrange("(k p) l -> p k l", p=128)

                    def emit_q_dma(qq):
                        l0 = qq * QL
                        xt = p1x.tile([128, KT, QL], BF, tag="xt")
                        nc.sync.dma_start(
                            out=xt[:, :, 0:QL // 2],
                            in_=xTr[:, :, l0:l0 + QL // 2])
                        nc.gpsimd.dma_start(
                            out=xt[:, :, QL // 2:QL],
                            in_=xTr[:, :, l0 + QL // 2:l0 + QL])
                        q_tiles[qq] = xt

                    def emit_q_comp(qq):
                        l0 = qq * QL
                        xt = q_tiles.pop(qq)
                        vTq = p1x.tile([128, QL], F, tag="vTq")
                        for ch in range(QL // 512):
                            proj(wk, kT, bk, xt, ch * 512, l0 + ch * 512)
                            proj(wv, vTq, None, xt, ch * 512, ch * 512)
                            if qq == 0:
                                proj(wq, qT, bq, xt, ch * 512, l0 + ch * 512)
                        for jl in range(QL // 128):
                            jt = qq * (QL // 128) + jl
                            pt = ps1.tile([128, 512], F, tag="ps1")
                            nc.tensor.transpose(
                                pt[:, 0:128],
                                vTq[:, jl * 128:(jl + 1) * 128], ident[:])
                            nc.vector.tensor_copy(vx0[:, jt, 0:DH], pt[:, 0:DH])
                            nc.vector.tensor_copy(vx1[:, jt, 0:DH],
                                                  pt[:, DH:D2])
                        if qq != 0:
                            for ch in range(QL // 512):
                                proj(wq, qT, bq, xt, ch * 512, l0 + ch * 512)

                    # chunk 0's attention interleaves into the remaining
                    # quarters so the in-order PE queue no longer serializes
                    # all of phase 1 ahead of the first scores matmul
                    emit_q_dma(0)
                    emit_q_comp(0)
                    emit_q_dma(1)
                    c0 = _phase2_chunk0(nc, (pss0, pss1), pv0, pv1, pe,
                                        p2v, qT, kT, vx0, vx1, NJ, F, FR,
                                        BF, I16, Act, Alu, emit_q_dma,
                                        emit_q_comp)

                if dbg:
                    nc.sync.dma_start(out=dbg_q.ap(), in_=qT[:].bitcast(F))
                    nc.sync.dma_start(out=dbg_k.ap(), in_=kT[:].bitcast(F))
                    nc.sync.dma_start(
                        out=dbg_v.ap(),
                        in_=vx0[:].bitcast(F).rearrange("p a b -> p (a b)"))

                # ---- Phase 2+3: attention, interleaved normalize/out-proj ----
                with tc.tile_pool(name="ps2o", bufs=2, space="PSUM") as ps2o:
                    _phase2(nc, (pss0, pss1), pv0, pv1, ps2o, pe, po, p2v,
                            qT, kT, vx0, vx1, wo0, wo1, out_d,
                            NI, NJ, F, FR, BF, I16, Act, Alu, c0)

    nc.compile()
    return nc


def _score_tile(tiles, n, NJ):
    return tiles[n % 2]


def _mk_emit_scores(nc, tiles, qT, kT, NJ):
    def emit_scores(n):
        ic, jt = n // NJ, n % NJ
        i0 = ic * 512
        j0 = jt * 128
        ps = _score_tile(tiles, n, NJ)
        nc.tensor.matmul(
            ps[:, 0:512], kT[0:64, j0:j0 + 128], qT[0:64, i0:i0 + 512],
            start=True, stop=True, tile_position=(0, 0),
        )
        nc.tensor.matmul(
            ps[:, 512:1024], kT[64:128, j0:j0 + 128],
            qT[64:128, i0:i0 + 512],
            start=True, stop=True, tile_position=(64, 0),
        )
    return emit_scores


def _mk_emit_exp(nc, tiles, NJ, BF, I16, Act, Alu):
    # Whole-tile exps alternating engines: per-op overhead (~0.2us pipe
    # drain + dispatch) makes one [128,1024] op per tile cheaper than
    # split halves, even though the halves would free the bank sooner.
    def emit_exp(n, jt, eT):
        ps = _score_tile(tiles, n, NJ)
        if jt % 2 == 1:
            nc.vector.tensor_scalar(
                eT[:].bitcast(I16), ps[:], SCH_A, SCH_B,
                Alu.mult, Alu.add)
        else:
            nc.scalar.activation(eT[:], ps[:], Act.Exp, scale=0.125)
    return emit_exp


def _mk_emit_chain(nc, pv0, pv1, vx0, vx1, NJ):
    """One 4-tile attn@v accumulation chain for one head.  Chains are
    spread ~evenly over the j-loop (the PE activity monitor re-throttles
    the clock if any ~3.4us window is mostly idle)."""
    def emit_chain(eTs, b0, head):
        pv, vx, c0 = ((pv0, vx0, 0) if head == 0 else (pv1, vx1, 512))
        for bjt in range(b0, b0 + 4):
            nc.tensor.matmul(
                pv[0:DH + 1, :], vx[:, bjt, :],
                eTs[bjt][:, c0:c0 + 512],
                start=(bjt == 0), stop=(bjt == NJ - 1),
            )
    return emit_chain


def _phase2(nc, stiles, pv0, pv1, ps2o, pe, po, p2v, qT, kT, vx0, vx1,
            wo0, wo1, out_d, NI, NJ, F, FR, BF, I16, Act, Alu, c0):
    emit_scores = _mk_emit_scores(nc, stiles, qT, kT, NJ)
    emit_exp = _mk_emit_exp(nc, stiles, NJ, BF, I16, Act, Alu)
    emit_chain = _mk_emit_chain(nc, pv0, pv1, vx0, vx1, NJ)

    def norm_unit(ic, p0, p1):
        # reciprocal + broadcast of the softmax denominators for chunk ic
        for (va, _), tg in ((p0, "0"), (p1, "1")):
            sh = p2v.tile([1, 512], F, tag="sh" + tg)
            rc = p2v.tile([1, 512], F, tag="rc" + tg)
            rb = p2v.tile([DH, 512], F, tag="rb" + tg)
            nc.sync.dma_start(out=sh[:], in_=va[DH:DH + 1, :].bitcast(F))
            nc.vector.reciprocal_approx_fast(out=rc[:], in_=sh[:])
            nc.gpsimd.partition_broadcast(rb[:], rc[:], channels=DH)
            nc.vector.tensor_mul(va[0:DH, :], va[0:DH, :], rb[:])

    def oproj_unit(ic, p0, p1, iw, fc, evac="s"):
        # one output-projection tile of chunk ic
        isl = slice(iw * 128, (iw + 1) * 128)
        r0 = ic * 512 + iw * 128
        f0 = fc * 512
        pso = ps2o.tile([128, 512], F, tag="pso")
        nc.tensor.matmul(
            pso[:], p0[0][0:DH, isl], wo0[:, f0:f0 + 512],
            start=True, stop=False,
        )
        nc.tensor.matmul(
            pso[:], p1[0][0:DH, isl], wo1[:, f0:f0 + 512],
            start=False, stop=True,
        )
        ot = po.tile([128, 512], F, tag="ot")
        if evac == "s":
            nc.scalar.copy(ot[:], pso[:])
        else:
            nc.vector.tensor_copy(ot[:], pso[:])
        nc.sync.dma_start(
            out=out_d.ap()[r0:r0 + 128, f0:f0 + 512], in_=ot[:])

    # attn@v chains at jts 5,6, 9,10, ..., 29,30 (b=(jt-5)//4, head par),
    # out-proj units of the previous chunk at the chain-free jts
    OPROJ_JTS = (7, 8, 11, 12, 15, 16, 19, 20)

    pending = c0["pending"]
    carry = c0["carry"]
    for ic in range(1, NI):
        units = []
        if pending is not None:
            pic, pp0, pp1 = pending[0], pending[1], pending[2]
            units = [(pic, pp0, pp1, iw, fc,
                      "s" if (iw * 2 + fc) % 2 == 0 else "v")
                     for iw in range(4) for fc in range(F_OUT // 512)]
        eTs = []
        for jt in range(NJ):
            n = ic * NJ + jt
            eT = pe.tile([128, 1024], BF, tag="eT")
            emit_exp(n, jt, eT)
            eTs.append(eT)
            if jt == 1 and carry is not None:
                carry(0)
            if jt == 2 and pending is not None:
                nc.vector.tensor_copy(pending[1][0][:], pv0[0:DH + 1, :])
            if jt == 3 and carry is not None:
                carry(1)
                carry = None
                nc.vector.tensor_copy(pending[2][0][:], pv1[0:DH + 1, :])
            if jt == 4 and pending is not None:
                norm_unit(pending[0], pending[1], pending[2])
            if jt >= 5 and (jt - 5) % 4 in (0, 1):
                emit_chain(eTs, 4 * ((jt - 5) // 4), (jt - 5) % 4)
            if jt in OPROJ_JTS and units:
                oproj_unit(*units.pop(0))
            if n + 1 < NI * NJ:
                emit_scores(n + 1)
        va0 = p2v.tile([DH + 1, 512], FR, tag="va0")
        va1 = p2v.tile([DH + 1, 512], FR, tag="va1")
        if ic < NI - 1:
            ceTs = eTs
            carry = (lambda head, e=ceTs: emit_chain(e, NJ - 4, head))
            pending = (ic, (va0, None), (va1, None))
        else:
            emit_chain(eTs, NJ - 4, 0)
            emit_chain(eTs, NJ - 4, 1)
            nc.vector.tensor_copy(va0[:], pv0[0:DH + 1, :])
            nc.vector.tensor_copy(va1[:], pv1[0:DH + 1, :])
            pending = (ic, (va0, None), (va1, None))

    norm_unit(pending[0], pending[1], pending[2])
    for iw in range(4):
        for fc in range(F_OUT // 512):
            oproj_unit(pending[0], pending[1], pending[2], iw, fc,
                       "s" if fc == 0 else "v")


def _phase2_chunk0(nc, stiles, pv0, pv1, pe, p2v, qT, kT, vx0, vx1, NJ,
                   F, FR, BF, I16, Act, Alu, emit_q_dma, emit_q_comp):
    """Chunk 0's j-loop, emitted inside the phase-1 pool scope with the
    remaining quarters' DMAs/compute interleaved at fixed j-slots."""
    emit_scores = _mk_emit_scores(nc, stiles, qT, kT, NJ)
    emit_exp = _mk_emit_exp(nc, stiles, NJ, BF, I16, Act, Alu)
    emit_chain = _mk_emit_chain(nc, pv0, pv1, vx0, vx1, NJ)

    eTs = []
    emit_scores(0)
    for jt in range(NJ):
        eT = pe.tile([128, 1024], BF, tag="eT")
        emit_exp(jt, jt, eT)
        eTs.append(eT)
        if jt >= 5 and (jt - 5) % 4 in (0, 1):
            emit_chain(eTs, 4 * ((jt - 5) // 4), (jt - 5) % 4)
        if jt % 8 == 6 and jt < NJ - 2:
            emit_q_comp(jt // 8 + 1)
            if jt // 8 + 2 < 4:
                emit_q_dma(jt // 8 + 2)
        emit_scores(jt + 1)
    va0 = p2v.tile([DH + 1, 512], FR, tag="va0")
    va1 = p2v.tile([DH + 1, 512], FR, tag="va1")
    carry = (lambda head, e=eTs: emit_chain(e, NJ - 4, head))
    return {"pending": (0, (va0, None), (va1, None)), "carry": carry}


def _get_built():
    global _BUILT
    if _BUILT is None:
        _BUILT = _build()
    return _BUILT


def kernel(x, Wq, bq, Wk, bk, Wv, bv, Wo, bo):
    from concourse.bass_utils import run_bass_kernel_spmd

    x = np.ascontiguousarray(np.asarray(x, dtype=np.float32))
    Wq = np.asarray(Wq, dtype=np.float32)
    Wk = np.asarray(Wk, dtype=np.float32)
    Wv = np.asarray(Wv, dtype=np.float32)
    Wo = np.asarray(Wo, dtype=np.float32)
    bq = np.asarray(bq, dtype=np.float32)
    bk = np.asarray(bk, dtype=np.float32)
    bv = np.asarray(bv, dtype=np.float32)
    bo = np.asarray(bo, dtype=np.float32)

    nc = _get_built()

    import ml_dtypes
    BFH = ml_dtypes.bfloat16
    xT = np.ascontiguousarray(x.T.astype(BFH))  # [F_IN, L] bf16
    in_maps = []
    for c in range(NCORES):
        hs = slice(c * HPC, (c + 1) * HPC)
        in_maps.append({
            "xT": xT,
            "wq": np.ascontiguousarray(
                Wq[:, hs, :].reshape(F_IN, D2).astype(BFH)),
            "wk": np.ascontiguousarray(
                Wk[:, hs, :].reshape(F_IN, D2).astype(BFH)),
            "wv": np.ascontiguousarray(
                Wv[:, hs, :].reshape(F_IN, D2).astype(BFH)),
            "bq": np.ascontiguousarray(bq[hs].reshape(D2)),
            "bk": np.ascontiguousarray(bk[hs].reshape(D2)),
            "wo0": np.ascontiguousarray(Wo[c * HPC]),
            "wo1": np.ascontiguousarray(Wo[c * HPC + 1]),
        })

    res = run_bass_kernel_spmd(nc, in_maps, list(range(NCORES)))
    acc = np.zeros((L, F_OUT), dtype=np.float64)
    for c in range(NCORES):
        acc += res.results[c]["out"].astype(np.float64)
    # bv contribution (softmax rows sum to 1) + bo, both exact on host
    acc += (bv.reshape(1, H * DH).astype(np.float64)
            @ Wo.reshape(H * DH, F_OUT).astype(np.float64))
    acc += bo.astype(np.float64)
    return acc.astype(np.float32)


# revision 28
# speedup vs baseline: 1.1232x; 1.0033x over previous
"""Multi-head self-attention Trainium2 kernel (8 NeuronCores, head-parallel).

Problem: L=4096, F_IN=1024, H=16, DH=64, F_OUT=1024, fp32.
Sharding: 2 heads per core (tensor parallel over heads). Each core computes
its 2 heads' attention and its partial output projection; the host sums the
8 partials (the all-reduce of the sharding hint, done at gather time).

Numerics: x and Wq/Wk/Wv are loaded in bf16; projections accumulate in fp32
PSUM. The attention matmuls run in bf16 with fp32 PSUM accumulation. Exps
alternate between ScalarE (exact ACT exp) and VectorE (Schraudolph
int16-bitcast approximation); the softmax denominator is summed from the
*rounded* attention weights (ones-column trick), so rounding largely
cancels in the normalization.

Schedule design (from perfetto analysis):
  - The binding constraint is a latency loop: scores(n+k) cannot issue
    until the PSUM bank of scores(n) is freed by exp(n). Phase 2 uses a
    persistent 5-bank scores arena with slots rotating mod 5 (2.5 tiles in
    flight) so the loop latency (scores + sem + exp + sem ~ 1.6us) divides
    by 2.5 and the PE becomes the pacer. Chunk 0 uses a 4-slot aligned
    rotation because phase 1 still holds 2 PSUM banks.
  - Per-jt emission order: exp first, then long-ready PE work (attn@v
    lagged 16 tiles, out-proj of the previous chunk), and the bank-gated
    scores LAST so a stall cannot head-of-line-block ready matmuls.
  - Chunk-end work (final attn@v block, pv evac, norm) is deferred into
    the next chunk's first iterations.
  - Input DMAs are spread across HW queues (sync/vector/gpsimd) and the
    x quarters are fetched half-first so the first k-projection can start
    as early as possible; a burst of dummy matmuls warms the PE HAM clock
    gate during the initial DMA wait.
"""

import numpy as np

L, F_IN, H, DH, F_OUT = 4096, 1024, 16, 64, 1024

# Schraudolph exp constants (DVE): int16(ps*SCH_A + SCH_B) bitcast bf16
SCH_C = 0.0579
SCH_A = 128.0 * 1.4426950408889634 * 0.125
SCH_B = 128.0 * (127.0 - SCH_C)
NCORES = 8
HPC = H // NCORES  # heads per core = 2
D2 = HPC * DH      # 128, per-core packed head dim

_BUILT = None


def _build():
    import os

    import concourse.bass as bass  # noqa: F401
    import concourse.mybir as mybir
    import concourse.tile as tile
    from concourse import bacc
    from concourse.masks import make_identity

    F = mybir.dt.float32
    FR = mybir.dt.float32r
    BF = mybir.dt.bfloat16
    I16 = mybir.dt.int16
    Alu = mybir.AluOpType
    Act = mybir.ActivationFunctionType

    nc = bacc.Bacc("TRN2", target_bir_lowering=False, debug=False)

    xT_d = nc.declare_dram_parameter("xT", [F_IN, L], BF, isOutput=False)
    wq_d = nc.declare_dram_parameter("wq", [F_IN, D2], BF, isOutput=False)
    wk_d = nc.declare_dram_parameter("wk", [F_IN, D2], BF, isOutput=False)
    wv_d = nc.declare_dram_parameter("wv", [F_IN, D2], BF, isOutput=False)
    bq_d = nc.declare_dram_parameter("bq", [D2], F, isOutput=False)
    bk_d = nc.declare_dram_parameter("bk", [D2], F, isOutput=False)
    wo0_d = nc.declare_dram_parameter("wo0", [DH, F_OUT], F, isOutput=False)
    wo1_d = nc.declare_dram_parameter("wo1", [DH, F_OUT], F, isOutput=False)
    out_d = nc.declare_dram_parameter("out", [L, F_OUT], F, isOutput=True)

    dbg = bool(os.environ.get("K_DEBUG"))
    if dbg:
        dbg_q = nc.declare_dram_parameter("dbg_q", [128, L], F, isOutput=True)
        dbg_k = nc.declare_dram_parameter("dbg_k", [128, L], F, isOutput=True)
        dbg_v = nc.declare_dram_parameter("dbg_v", [128, 32 * 65], F, isOutput=True)

    KT = F_IN // 128   # 8 f-tiles
    NI = L // 512      # 8 i-chunks
    NJ = L // 128      # 32 j-tiles
    QL = 1024          # quarter width in L
    NQ = L // QL       # 4 quarters

    with tile.TileContext(nc) as tc:
        with tc.tile_pool(name="persist", bufs=1) as pp:
            qT = pp.tile([128, L], BF, tag="qT")             # [d2, i]
            kT = pp.tile([128, L], BF, tag="kT")             # [d2, j]
            vx0 = pp.tile([128, NJ, DH + 1], BF, tag="vx0")  # [j_in, jt, d|1]
            vx1 = pp.tile([128, NJ, DH + 1], BF, tag="vx1")
            bq = pp.tile([128, 1], F, tag="bq")
            bk = pp.tile([128, 1], F, tag="bk")
            ones32 = pp.tile([128, NJ], F, tag="ones32")
            warm = pp.tile([1, 1], F, tag="warm")
            wmm = pp.tile([64, 128], BF, tag="wmm")

            # pre-warm the exp table set while DMAs run
            nc.vector.memset(warm[:], 0.0)
            nc.scalar.activation(warm[:], warm[:], Act.Exp, scale=1.0)

            nc.vector.memset(wmm[:], 0.0)
            nc.vector.memset(ones32[:], 1.0)
            nc.vector.tensor_copy(vx0[:, :, DH:DH + 1], ones32[:, :, None])
            nc.vector.tensor_copy(vx1[:, :, DH:DH + 1], ones32[:, :, None])

            # Pools for the attention phase are opened before phase 1 is
            # emitted so the scheduler can overlap the phase-1 tail with
            # early score matmuls (PSUM: arena 5 + pv 2 + {ps1 2 during
            # phase1 / pso 1 after} = 8; chunk 0 only touches arena
            # slots 0-3).
            with tc.tile_pool(name="p2", bufs=1) as p2, \
                 tc.tile_pool(name="p2v", bufs=2) as p2v, \
                 tc.tile_pool(name="expp", bufs=20) as pe, \
                 tc.tile_pool(name="outp", bufs=4) as po, \
                 tc.tile_pool(name="ps2s", bufs=1, space="PSUM") as ps2s, \
                 tc.tile_pool(name="ps2v", bufs=1, space="PSUM") as ps2v:
                pss0 = ps2s.tile([128, 1024], F, tag="pss0")
                pss1 = ps2s.tile([128, 1024], F, tag="pss1")
                pv0 = ps2v.tile([128, 512], F, tag="pv0")
                pv1 = ps2v.tile([128, 512], F, tag="pv1")
                wo0 = p2.tile([DH, F_OUT], FR, tag="wo0")
                wo1 = p2.tile([DH, F_OUT], FR, tag="wo1")

                # ---- Phase 1: QKV projections over 4 quarters of L ----
                with tc.tile_pool(name="p1w", bufs=1) as p1w, \
                     tc.tile_pool(name="p1x", bufs=2) as p1x, \
                     tc.tile_pool(name="ps1", bufs=2, space="PSUM") as ps1:
                    wq = p1w.tile([128, KT, D2], BF, tag="wq")
                    wk = p1w.tile([128, KT, D2], BF, tag="wk")
                    wv = p1w.tile([128, KT, D2], BF, tag="wv")
                    ident = p1w.tile([128, 128], F, tag="ident")
                    for wt, wd in ((wk, wk_d), (wv, wv_d), (wq, wq_d)):
                        nc.scalar.dma_start(
                            out=wt[:],
                            in_=wd.ap().rearrange("(k p) d -> p k d", p=128),
                        )
                    make_identity(nc, ident[:])
                    nc.scalar.dma_start(out=bq[:], in_=bq_d.ap()[:, None])
                    nc.scalar.dma_start(out=bk[:], in_=bk_d.ap()[:, None])
                    nc.scalar.dma_start(out=wo0[:],
                                        in_=wo0_d.ap().bitcast(FR))
                    nc.scalar.dma_start(out=wo1[:],
                                        in_=wo1_d.ap().bitcast(FR))

                    # HAM warm-up: ~4us of dummy matmuls during the input
                    # DMA wait so the real projections run at 2.4 GHz
                    psw = ps1.tile([128, 512], F, tag="ps1")
                    for _ in range(36):
                        nc.tensor.matmul(
                            psw[0:64, 0:128], wmm[:, 0:64], wmm[:, :],
                            start=True, stop=True,
                        )

                    def proj(wt, dst, bias, xt, c0, g0):
                        ps = ps1.tile([128, 512], F, tag="ps1")
                        for kt in range(KT):
                            nc.tensor.matmul(
                                ps[:], wt[:, kt, :], xt[:, kt, c0:c0 + 512],
                                start=(kt == 0), stop=(kt == KT - 1),
                            )
                        if bias is not None:
                            nc.scalar.activation(
                                dst[:, g0:g0 + 512], ps[:], Act.Identity,
                                bias=bias[:], scale=1.0,
                            )
                        else:
                            nc.scalar.copy(dst[:, c0:c0 + 512], ps[:])

                    q_tiles = {}

                    xTr = xT_d.ap().rearrange("(k p) l -> p k l", p=128)

                    def emit_q_dma(qq):
                        l0 = qq * QL
                        xt = p1x.tile([128, KT, QL], BF, tag="xt")
                        nc.sync.dma_start(
                            out=xt[:, :, 0:QL // 2],
                            in_=xTr[:, :, l0:l0 + QL // 2])
                        nc.gpsimd.dma_start(
                            out=xt[:, :, QL // 2:QL],
                            in_=xTr[:, :, l0 + QL // 2:l0 + QL])
                        q_tiles[qq] = xt

                    def emit_q_comp(qq):
                        l0 = qq * QL
                        xt = q_tiles.pop(qq)
                        vTq = p1x.tile([128, QL], F, tag="vTq")
                        for ch in range(QL // 512):
                            proj(wk, kT, bk, xt, ch * 512, l0 + ch * 512)
                            proj(wv, vTq, None, xt, ch * 512, ch * 512)
                            if qq == 0:
                                proj(wq, qT, bq, xt, ch * 512, l0 + ch * 512)
                        for jl in range(QL // 128):
                            jt = qq * (QL // 128) + jl
                            pt = ps1.tile([128, 512], F, tag="ps1")
                            nc.tensor.transpose(
                                pt[:, 0:128],
                                vTq[:, jl * 128:(jl + 1) * 128], ident[:])
                            nc.vector.tensor_copy(vx0[:, jt, 0:DH], pt[:, 0:DH])
                            nc.vector.tensor_copy(vx1[:, jt, 0:DH],
                                                  pt[:, DH:D2])
                        if qq != 0:
                            for ch in range(QL // 512):
                                proj(wq, qT, bq, xt, ch * 512, l0 + ch * 512)

                    # chunk 0's attention interleaves into the remaining
                    # quarters so the in-order PE queue no longer serializes
                    # all of phase 1 ahead of the first scores matmul
                    emit_q_dma(0)
                    emit_q_comp(0)
                    emit_q_dma(1)
                    c0 = _phase2_chunk0(nc, (pss0, pss1), pv0, pv1, pe,
                                        p2v, qT, kT, vx0, vx1, NJ, F, FR,
                                        BF, I16, Act, Alu, emit_q_dma,
                                        emit_q_comp)

                if dbg:
                    nc.sync.dma_start(out=dbg_q.ap(), in_=qT[:].bitcast(F))
                    nc.sync.dma_start(out=dbg_k.ap(), in_=kT[:].bitcast(F))
                    nc.sync.dma_start(
                        out=dbg_v.ap(),
                        in_=vx0[:].bitcast(F).rearrange("p a b -> p (a b)"))

                # ---- Phase 2+3: attention, interleaved normalize/out-proj ----
                with tc.tile_pool(name="ps2o", bufs=2, space="PSUM") as ps2o:
                    _phase2(nc, (pss0, pss1), pv0, pv1, ps2o, pe, po, p2v,
                            qT, kT, vx0, vx1, wo0, wo1, out_d,
                            NI, NJ, F, FR, BF, I16, Act, Alu, c0)

    nc.compile()
    return nc


def _score_tile(tiles, n, NJ):
    return tiles[n % 2]


def _mk_emit_scores(nc, tiles, qT, kT, NJ):
    def emit_scores(n):
        ic, jt = n // NJ, n % NJ
        i0 = ic * 512
        j0 = jt * 128
        ps = _score_tile(tiles, n, NJ)
        nc.tensor.matmul(
            ps[:, 0:512], kT[0:64, j0:j0 + 128], qT[0:64, i0:i0 + 512],
            start=True, stop=True, tile_position=(0, 0),
        )
        nc.tensor.matmul(
            ps[:, 512:1024], kT[64:128, j0:j0 + 128],
            qT[64:128, i0:i0 + 512],
            start=True, stop=True, tile_position=(64, 0),
        )
    return emit_scores


def _mk_emit_exp(nc, tiles, NJ, BF, I16, Act, Alu):
    # Whole-tile exps alternating engines: per-op overhead (~0.2us pipe
    # drain + dispatch) makes one [128,1024] op per tile cheaper than
    # split halves, even though the halves would free the bank sooner.
    def emit_exp(n, jt, eT):
        ps = _score_tile(tiles, n, NJ)
        if jt % 2 == 1:
            nc.vector.tensor_scalar(
                eT[:].bitcast(I16), ps[:], SCH_A, SCH_B,
                Alu.mult, Alu.add)
        else:
            nc.scalar.activation(eT[:], ps[:], Act.Exp, scale=0.125)
    return emit_exp


def _mk_emit_chain(nc, pv0, pv1, vx0, vx1, NJ):
    """One 4-tile attn@v accumulation chain for one head.  Chains are
    spread ~evenly over the j-loop (the PE activity monitor re-throttles
    the clock if any ~3.4us window is mostly idle)."""
    def emit_chain(eTs, b0, head):
        pv, vx, c0 = ((pv0, vx0, 0) if head == 0 else (pv1, vx1, 512))
        for bjt in range(b0, b0 + 4):
            nc.tensor.matmul(
                pv[0:DH + 1, :], vx[:, bjt, :],
                eTs[bjt][:, c0:c0 + 512],
                start=(bjt == 0), stop=(bjt == NJ - 1),
            )
    return emit_chain


def _phase2(nc, stiles, pv0, pv1, ps2o, pe, po, p2v, qT, kT, vx0, vx1,
            wo0, wo1, out_d, NI, NJ, F, FR, BF, I16, Act, Alu, c0):
    emit_scores = _mk_emit_scores(nc, stiles, qT, kT, NJ)
    emit_exp = _mk_emit_exp(nc, stiles, NJ, BF, I16, Act, Alu)
    emit_chain = _mk_emit_chain(nc, pv0, pv1, vx0, vx1, NJ)

    def norm_unit(ic, p0, p1):
        # reciprocal + broadcast of the softmax denominators for chunk ic
        for (va, _), tg in ((p0, "0"), (p1, "1")):
            sh = p2v.tile([1, 512], F, tag="sh" + tg)
            rc = p2v.tile([1, 512], F, tag="rc" + tg)
            rb = p2v.tile([DH, 512], F, tag="rb" + tg)
            nc.sync.dma_start(out=sh[:], in_=va[DH:DH + 1, :].bitcast(F))
            nc.vector.reciprocal_approx_fast(out=rc[:], in_=sh[:])
            nc.gpsimd.partition_broadcast(rb[:], rc[:], channels=DH)
            nc.vector.tensor_mul(va[0:DH, :], va[0:DH, :], rb[:])

    def oproj_unit(ic, p0, p1, iw, fc, evac="s", dmae=None):
        # one output-projection tile of chunk ic
        isl = slice(iw * 128, (iw + 1) * 128)
        r0 = ic * 512 + iw * 128
        f0 = fc * 512
        pso = ps2o.tile([128, 512], F, tag="pso")
        nc.tensor.matmul(
            pso[:], p0[0][0:DH, isl], wo0[:, f0:f0 + 512],
            start=True, stop=False,
        )
        nc.tensor.matmul(
            pso[:], p1[0][0:DH, isl], wo1[:, f0:f0 + 512],
            start=False, stop=True,
        )
        ot = po.tile([128, 512], F, tag="ot")
        if evac == "s":
            nc.scalar.copy(ot[:], pso[:])
        else:
            nc.vector.tensor_copy(ot[:], pso[:])
        (dmae or nc.sync).dma_start(
            out=out_d.ap()[r0:r0 + 128, f0:f0 + 512], in_=ot[:])

    # attn@v chains at jts 5,6, 9,10, ..., 29,30 (b=(jt-5)//4, head par),
    # out-proj units of the previous chunk at the chain-free jts
    OPROJ_JTS = (7, 8, 11, 12, 15, 16, 19, 20)

    pending = c0["pending"]
    carry = c0["carry"]
    for ic in range(1, NI):
        units = []
        if pending is not None:
            pic, pp0, pp1 = pending[0], pending[1], pending[2]
            units = [(pic, pp0, pp1, iw, fc,
                      "s" if (iw * 2 + fc) % 2 == 0 else "v")
                     for iw in range(4) for fc in range(F_OUT // 512)]
        eTs = []
        for jt in range(NJ):
            n = ic * NJ + jt
            eT = pe.tile([128, 1024], BF, tag="eT")
            emit_exp(n, jt, eT)
            eTs.append(eT)
            if jt == 1 and carry is not None:
                carry(0)
            if jt == 2 and pending is not None:
                nc.vector.tensor_copy(pending[1][0][:], pv0[0:DH + 1, :])
            if jt == 3 and carry is not None:
                carry(1)
                carry = None
                nc.vector.tensor_copy(pending[2][0][:], pv1[0:DH + 1, :])
            if jt == 4 and pending is not None:
                norm_unit(pending[0], pending[1], pending[2])
            if jt >= 5 and (jt - 5) % 4 in (0, 1):
                emit_chain(eTs, 4 * ((jt - 5) // 4), (jt - 5) % 4)
            if jt in OPROJ_JTS and units:
                oproj_unit(*units.pop(0))
            if n + 1 < NI * NJ:
                emit_scores(n + 1)
        va0 = p2v.tile([DH + 1, 512], FR, tag="va0")
        va1 = p2v.tile([DH + 1, 512], FR, tag="va1")
        if ic < NI - 1:
            ceTs = eTs
            carry = (lambda head, e=ceTs: emit_chain(e, NJ - 4, head))
            pending = (ic, (va0, None), (va1, None))
        else:
            emit_chain(eTs, NJ - 4, 0)
            emit_chain(eTs, NJ - 4, 1)
            nc.vector.tensor_copy(va0[:], pv0[0:DH + 1, :])
            nc.vector.tensor_copy(va1[:], pv1[0:DH + 1, :])
            pending = (ic, (va0, None), (va1, None))

    # tail: the final chunk's projection drains 2MB — split the DMAs
    # across the sync and scalar queues so the drain halves
    norm_unit(pending[0], pending[1], pending[2])
    for iw in range(4):
        for fc in range(F_OUT // 512):
            oproj_unit(pending[0], pending[1], pending[2], iw, fc,
                       "s" if fc == 0 else "v",
                       nc.sync if fc == 0 else nc.scalar)


def _phase2_chunk0(nc, stiles, pv0, pv1, pe, p2v, qT, kT, vx0, vx1, NJ,
                   F, FR, BF, I16, Act, Alu, emit_q_dma, emit_q_comp):
    """Chunk 0's j-loop, emitted inside the phase-1 pool scope with the
    remaining quarters' DMAs/compute interleaved at fixed j-slots."""
    emit_scores = _mk_emit_scores(nc, stiles, qT, kT, NJ)
    emit_exp = _mk_emit_exp(nc, stiles, NJ, BF, I16, Act, Alu)
    emit_chain = _mk_emit_chain(nc, pv0, pv1, vx0, vx1, NJ)

    eTs = []
    emit_scores(0)
    for jt in range(NJ):
        eT = pe.tile([128, 1024], BF, tag="eT")
        emit_exp(jt, jt, eT)
        eTs.append(eT)
        if jt >= 5 and (jt - 5) % 4 in (0, 1):
            emit_chain(eTs, 4 * ((jt - 5) // 4), (jt - 5) % 4)
        if jt % 8 == 6 and jt < NJ - 2:
            emit_q_comp(jt // 8 + 1)
            if jt // 8 + 2 < 4:
                emit_q_dma(jt // 8 + 2)
        emit_scores(jt + 1)
    va0 = p2v.tile([DH + 1, 512], FR, tag="va0")
    va1 = p2v.tile([DH + 1, 512], FR, tag="va1")
    carry = (lambda head, e=eTs: emit_chain(e, NJ - 4, head))
    return {"pending": (0, (va0, None), (va1, None)), "carry": carry}


def _get_built():
    global _BUILT
    if _BUILT is None:
        _BUILT = _build()
    return _BUILT


def kernel(x, Wq, bq, Wk, bk, Wv, bv, Wo, bo):
    from concourse.bass_utils import run_bass_kernel_spmd

    x = np.ascontiguousarray(np.asarray(x, dtype=np.float32))
    Wq = np.asarray(Wq, dtype=np.float32)
    Wk = np.asarray(Wk, dtype=np.float32)
    Wv = np.asarray(Wv, dtype=np.float32)
    Wo = np.asarray(Wo, dtype=np.float32)
    bq = np.asarray(bq, dtype=np.float32)
    bk = np.asarray(bk, dtype=np.float32)
    bv = np.asarray(bv, dtype=np.float32)
    bo = np.asarray(bo, dtype=np.float32)

    nc = _get_built()

    import ml_dtypes
    BFH = ml_dtypes.bfloat16
    xT = np.ascontiguousarray(x.T.astype(BFH))  # [F_IN, L] bf16
    in_maps = []
    for c in range(NCORES):
        hs = slice(c * HPC, (c + 1) * HPC)
        in_maps.append({
            "xT": xT,
            "wq": np.ascontiguousarray(
                Wq[:, hs, :].reshape(F_IN, D2).astype(BFH)),
            "wk": np.ascontiguousarray(
                Wk[:, hs, :].reshape(F_IN, D2).astype(BFH)),
            "wv": np.ascontiguousarray(
                Wv[:, hs, :].reshape(F_IN, D2).astype(BFH)),
            "bq": np.ascontiguousarray(bq[hs].reshape(D2)),
            "bk": np.ascontiguousarray(bk[hs].reshape(D2)),
            "wo0": np.ascontiguousarray(Wo[c * HPC]),
            "wo1": np.ascontiguousarray(Wo[c * HPC + 1]),
        })

    res = run_bass_kernel_spmd(nc, in_maps, list(range(NCORES)))
    acc = np.zeros((L, F_OUT), dtype=np.float64)
    for c in range(NCORES):
        acc += res.results[c]["out"].astype(np.float64)
    # bv contribution (softmax rows sum to 1) + bo, both exact on host
    acc += (bv.reshape(1, H * DH).astype(np.float64)
            @ Wo.reshape(H * DH, F_OUT).astype(np.float64))
    acc += bo.astype(np.float64)
    return acc.astype(np.float32)
